# revision 5
# baseline (speedup 1.0000x reference)
"""Trainium2 Bass kernel: Wan-style interleaved RoPE on q/k + causal attention.

Full problem: q,k,v [B=2, S=2048, H=16, D=128] fp32, freqs [1, S, 1, D].
  rq = rope(q), rk = rope(k)
  out[b,h,q,d] = softmax_causal(rq @ rk^T / sqrt(D)) @ v      -> [B, H, S, D]

Sharding: heads split across 8 cores (2 heads/core); each core handles
4 independent (b, h) attention problems. Inputs are sliced on host, the
SPMD kernel runs on cores 0-7, outputs are concatenated on host.

Layout trick: scores = sum_d rq[d]*rk[d] are invariant under any shared
permutation of d, so q and k are shipped de-interleaved (evens then
odds), pre-transposed to [D', S], AND pre-duplicated: the vector
engines have no cross-partition path, so each shipped tensor is
[128, 2S] = (E | O) with E = (x0|x0), O = (x1|x1) stacked so that with
FF = (f0|f1), GG = (-f1|f0):
  rqT' = E*FF + O*GG    -- exactly interleaved RoPE in (evens|odds)
                           d-order.
One fused DMA loads (E|O) per tensor (same bytes as the four half-DMAs
it replaces -- the duplication is in DRAM -- but 4x fewer descriptors).

Everything is shipped and computed in fp16.  Attention per (b,h), per
q-block of 512: k-tiles are bin-packed into 512-col PSUM banks, three
banks per [128, 1536] score tile, diagonal tiles reordered j0,j1,j3,j2
so the packing has no holes; one exp(s*scale - 8) per batch (the
uniform bias cancels in the normalization and keeps exp in fp16 range).

Softmax sums and normalization are finished on the HOST: the kernel
merges each q-block's probT tiles with an in-place wide tree on DVE --
q-aligned full batches are added [1536]-wide at 2x perf mode into batch
0's tile, misaligned diagonal tiles are added at their q-offsets into
segment 0 -- and ships batch 0 raw ([128,1536]; [128,512] for qb0).
The host folds the 512-col segments, reduces the 128 partitions,
and divides the unnormalized output (outT evacuated fp32->fp16 by
ScalarE into out_full [d, q]) by the sums.

Engine balance: q RoPE (2 mul + add) and k's add on DVE; k's two muls
on GpSimd; exp + outT evac on ScalarE; out/acc DMAs issue from GpSimd,
loads from Sync.  Load DMAs are staggered (k at attention start, q
after qb0, v after qb2) and the next (b,h)'s RoPE is emitted in two
stages (k muls after qb1, the DVE ops after qb2 -- so the DVE add
gated on GpSimd never blocks the in-order vector queue) to smooth the
DMA/power bursts that otherwise stall DVE ops 10-20x.

Boot: (b,h)=0's q/k are shipped a second time chunk-tiled [4, 128,
1024] ((E|O) per 512-col chunk, contiguous in DRAM) so boot loads+RoPE
run chunk-by-chunk and qb0's matmuls start early; freqs/v issue from
the otherwise-idle Scalar queue.
"""

import math

import numpy as np

B, S, H, D = 2, 2048, 16, 128
NCORES = 8
HPC = H // NCORES          # heads per core
NBH = B * HPC              # (b, h) problems per core
NT = S // 128              # s-tiles
QB = S // 512              # q blocks of 512
SCALE = 1.0 / math.sqrt(D)
NEG = -1e30
EXPBIAS = 8.0              # uniform softmax shift; keeps exp in fp16 range
SCW = 1536                 # packed score-tile width (3 PSUM banks)

_CACHE = {}


def _plan(qb):
    """Pack this q-block's k-tiles into contiguous score batches.

    A matmul output must not cross a 512-col PSUM bank boundary, so tiles
    are bin-packed into 512-col banks (3 banks per [128, SCW] score
    tile).  The diagonal tiles (widths 512/384/256/128) are emitted in
    the order j0, j1, j3, j2 so banks fill exactly ([512], [384+128],
    [256]) with no holes: each batch's valid columns are contiguous from
    0 and one exp instruction covers them.  The first tile (tk=0, full
    width) stays first so its start=True matmul resets every PSUM cell
    of the PV accumulator.

    Returns (nk, batches); each batch is a list of (tk, off, lo, w).
    Tiles with off == 0 always land at 512-aligned lo (bank starts), so
    the leading off==0 run of every batch is q-aligned for wide adds.
    """
    nk = 4 * qb + 4
    order = list(range(4 * qb)) + [4 * qb, 4 * qb + 1, 4 * qb + 3, 4 * qb + 2]
    batches, cur = [], []
    bank, used = 0, 0
    for tk in order:
        j = tk - 4 * qb
        off = 128 * j if j > 0 else 0
        w = 512 - off
        if used + w > 512:
            bank, used = bank + 1, 0
        if bank == SCW // 512:
            batches.append(cur)
            cur, bank = [], 0
        cur.append((tk, off, bank * 512 + used, w))
        used += w
    batches.append(cur)
    return nk, batches


def _build():
    import concourse.mybir as mybir
    import concourse.tile as tile
    from concourse import bacc
    from concourse.masks import make_identity

    f32 = mybir.dt.float32
    f16 = mybir.dt.float16
    bf16 = mybir.dt.bfloat16
    Alu = mybir.AluOpType
    Act = mybir.ActivationFunctionType

    nc = bacc.Bacc("TRN2", target_bir_lowering=False, debug=False,
                   num_devices=NCORES)
    # steady-state loads: fused (E|O) [128, 2S] per (b,h)
    qd = nc.dram_tensor("qEO", [NBH, 128, 2 * S], f16, kind="ExternalInput")
    kd = nc.dram_tensor("kEO", [NBH, 128, 2 * S], f16, kind="ExternalInput")
    vd = nc.dram_tensor("v", [NBH, 128, S], f16, kind="ExternalInput")
    # boot copies of (b,h)=0's q/k, chunk-tiled [4, 128, 1024] ((E|O) per
    # 512-col chunk, contiguous): boot loads+RoPE run chunk-by-chunk.
    qbd = nc.dram_tensor("qEOb", [4, 128, 1024], f16, kind="ExternalInput")
    kbd = nc.dram_tensor("kEOb", [4, 128, 1024], f16, kind="ExternalInput")
    fd = nc.dram_tensor("freqsT", [4, D, 512], f16, kind="ExternalInput")
    gd = nc.dram_tensor("freqsG", [4, D, 512], f16, kind="ExternalInput")
    od = nc.dram_tensor("out", [NBH, 128, S], f16, kind="ExternalOutput")
    ad = nc.dram_tensor("accs", [NBH, QB, 128, SCW], f16,
                        kind="ExternalOutput")

    with tile.TileContext(nc) as tc:
        with (
            tc.tile_pool(name="const", bufs=1) as cpool,
            tc.tile_pool(name="io", bufs=2) as iopool,
            tc.tile_pool(name="rope", bufs=2) as rpool,
            tc.tile_pool(name="xt", bufs=2) as xtpool,
            tc.tile_pool(name="prob", bufs=8) as ppool,
            tc.tile_pool(name="outf", bufs=2) as opool,
            tc.tile_pool(name="sc_ps", bufs=2, space="PSUM") as sc_ps,
            tc.tile_pool(name="out_ps", bufs=2, space="PSUM") as out_ps,
        ):
            # ---- constants ----
            ident = cpool.tile([128, 128], f32, tag="ident")
            make_identity(nc, ident[:])
            # tri_bf[k, t] = 0 where k <= t (valid), NEG where k > t.
            tri_bf = cpool.tile([128, 128], bf16, tag="tri_bf")
            nc.gpsimd.memset(tri_bf[:], 0.0)
            nc.gpsimd.affine_select(
                out=tri_bf[:], in_=tri_bf[:],
                compare_op=Alu.is_ge, fill=NEG, base=0,
                pattern=[[1, 128]], channel_multiplier=-1,
            )
            ident_bf = cpool.tile([128, 128], bf16, tag="ident_bf")
            nc.vector.tensor_copy(ident_bf[:], ident[:])
            nbias = cpool.tile([128, 1], f32, tag="nbias")
            nc.vector.memset(nbias[:], -EXPBIAS)
            FF = cpool.tile([128, S], f16, tag="FF")
            GG = cpool.tile([128, S], f16, tag="GG")

            def rope_compute(xEO, xT_ap, mul_eng, add_eng, cs=slice(0, S)):
                n = cs.stop - cs.start
                xE = xEO[:, 0:n]
                xO = xEO[:, n:2 * n]
                mul_eng.tensor_mul(xE, xE, FF[:, cs])
                mul_eng.tensor_mul(xO, xO, GG[:, cs])
                add_eng.tensor_add(xT_ap, xE, xO)

            def emit_load_boot():
                """Chunked load+RoPE for (b,h)=0 from the chunk-tiled boot
                tensors: qb0's matmuls start as soon as chunk 0 lands.
                k's muls on GpSimd, q's on DVE (they run in parallel);
                freqs/v issue from the otherwise-idle Scalar queue."""
                qTc = [xtpool.tile([128, 512], f16, tag=f"bqT{c}",
                                   name=f"bqT{c}", bufs=1) for c in range(4)]
                kTc = [xtpool.tile([128, 512], f16, tag=f"bkT{c}",
                                   name=f"bkT{c}", bufs=1) for c in range(4)]
                for c in range(4):
                    cs = slice(c * 512, (c + 1) * 512)
                    nc.scalar.dma_start(FF[:, cs], fd.ap()[c])
                    nc.scalar.dma_start(GG[:, cs], gd.ap()[c])
                    kEO = rpool.tile([128, 1024], f16, tag="bkEO",
                                     name="bkEO")
                    nc.sync.dma_start(kEO[:], kbd.ap()[c])
                    rope_compute(kEO, kTc[c][:], nc.gpsimd, nc.vector, cs)
                    qEO = rpool.tile([128, 1024], f16, tag="bqEO",
                                     name="bqEO")
                    nc.gpsimd.dma_start(qEO[:], qbd.ap()[c])
                    rope_compute(qEO, qTc[c][:], nc.vector, nc.vector, cs)
                    if c == 0:
                        v_mm = iopool.tile([128, S], f16, tag="v_mm",
                                           name="v_mm")
                        nc.scalar.dma_start(v_mm[:], vd.ap()[0])

                def kT_lhsT(tk):
                    return kTc[tk // 4][:, (tk % 4) * 128:(tk % 4 + 1) * 128]

                def qT_rhs(qb, off):
                    return qTc[qb][:, off:512]

                return (kT_lhsT, qT_rhs, v_mm)

            def emit_attention(bh, acc, hooks):
                kT_lhsT, qT_rhs, v_mm = acc
                last = bh == NBH - 1
                out_full = opool.tile([128, S], f16, tag="out_full",
                                      name="out_full")

                def phase_compute(qb):
                    nk, batches = _plan(qb)
                    last_tk = batches[-1][-1][0]
                    outs = out_full[:, qb * 512:(qb + 1) * 512]
                    outT = out_ps.tile([128, 512], f32, tag="outT",
                                       name="outT")
                    probts = []
                    na0 = sum(1 for t in batches[0] if t[1] == 0)
                    for batch in batches:
                        sc = sc_ps.tile([128, SCW], f32, tag="sc", name="sc")
                        for tk, off, lo, w in batch:
                            diag = tk >= 4 * qb
                            nc.tensor.matmul(
                                sc[:, lo:lo + w], kT_lhsT(tk),
                                qT_rhs(qb, off),
                                start=True, stop=not diag,
                            )
                            if diag:
                                nc.tensor.matmul(
                                    sc[:, lo:lo + 128],
                                    ident_bf[:], tri_bf[:],
                                    start=False, stop=True,
                                )
                        wtot = batch[-1][2] + batch[-1][3]
                        probt = ppool.tile([128, SCW], f16, tag="probt",
                                           name="probt")
                        nc.scalar.activation(
                            probt[:, 0:wtot], sc[:, 0:wtot],
                            Act.Exp, scale=SCALE, bias=nbias[:],
                        )
                        for tk, off, lo, w in batch:
                            nc.tensor.matmul(
                                outT[:, off:512],
                                v_mm[:, tk * 128:(tk + 1) * 128],
                                probt[:, lo:lo + w],
                                start=(tk == 0), stop=(tk == last_tk),
                            )
                        probts.append(probt)
                        # wide-tree merge: as soon as this batch's probt is
                        # final, add its q-aligned prefix (leading off==0
                        # tiles, always at 512-aligned lo) into batch 0's
                        # tile in place, [<=1536]-wide at DVE 2x rate.
                        if len(probts) > 1:
                            na = 0
                            for t in batch:
                                if t[1] != 0:
                                    break
                                na += 1
                            m = 512 * min(na, na0)
                            if m:
                                nc.vector.tensor_add(
                                    probts[0][:, 0:m], probts[0][:, 0:m],
                                    probt[:, 0:m])
                    # fold the misaligned diagonal tiles into batch 0's
                    # segment 0 at their q-offsets; the host folds the
                    # 512-col segments and reduces the 128 partitions.
                    p0 = probts[0]
                    for bi, batch in enumerate(batches):
                        seen_mis = False
                        for tk, off, lo, w in batch:
                            if off == 0 and not seen_mis:
                                continue
                            seen_mis = True
                            nc.vector.tensor_add(
                                p0[:, off:512], p0[:, off:512],
                                probts[bi][:, lo:lo + w])
                    nc.gpsimd.dma_start(
                        ad.ap()[bh, qb, :, 0:512 * na0],
                        p0[:, 0:512 * na0])
                    # evacuate the unnormalized PV accumulator fp32->fp16 on
                    # ScalarE (its queue holds only exps); host divides by
                    # the sums.
                    nc.scalar.copy(outs, outT[:])
                    if last:
                        nc.gpsimd.dma_start(
                            od.ap()[bh, :, qb * 512:(qb + 1) * 512], outs)

                for qb in range(QB):
                    phase_compute(qb)
                    for fn in hooks.get(qb, ()):
                        fn()
                if not last:
                    nc.gpsimd.dma_start(od.ap()[bh], out_full[:])

            def make_hooks(bh, accs, state):
                """Stagger next-(b,h) loads and RoPE across this (b,h)'s
                q-blocks to smooth DMA/power bursts:
                  qb0 end: k load     qb1 end: q load + k RoPE muls (GpSimd)
                  qb2 end: k RoPE add + q RoPE (DVE) + v load
                (k load issues at attention start, before qb0.)"""
                nxt = bh + 1
                if nxt >= NBH:
                    return {}

                def load_k():
                    kEO = rpool.tile([128, 2 * S], f16, tag="kEO",
                                     name="kEO")
                    nc.sync.dma_start(kEO[:], kd.ap()[nxt])
                    state["kEO"] = kEO

                def load_q():
                    qEO = rpool.tile([128, 2 * S], f16, tag="qEO",
                                     name="qEO")
                    nc.sync.dma_start(qEO[:], qd.ap()[nxt])
                    state["qEO"] = qEO

                def rope_k_muls():
                    kT = xtpool.tile([128, S], f16, tag="kT", name="kT")
                    kEO = state["kEO"]
                    nc.gpsimd.tensor_mul(kEO[:, 0:S], kEO[:, 0:S], FF[:])
                    nc.gpsimd.tensor_mul(kEO[:, S:2 * S], kEO[:, S:2 * S],
                                         GG[:])
                    state["kT"] = kT

                def rope_rest():
                    kEO, qEO = state["kEO"], state["qEO"]
                    kT, qT = state["kT"], xtpool.tile([128, S], f16,
                                                      tag="qT", name="qT")
                    nc.vector.tensor_add(kT[:], kEO[:, 0:S], kEO[:, S:2 * S])
                    rope_compute(qEO, qT[:], nc.vector, nc.vector)
                    v_mm = iopool.tile([128, S], f16, tag="v_mm",
                                       name="v_mm")
                    nc.sync.dma_start(v_mm[:], vd.ap()[nxt])

                    def kT_lhsT(tk):
                        return kT[:, tk * 128:(tk + 1) * 128]

                    def qT_rhs(qb, off):
                        return qT[:, qb * 512 + off:(qb + 1) * 512]

                    accs[nxt] = (kT_lhsT, qT_rhs, v_mm)

                if bh == 0:
                    # don't compete with the latency-critical boot chunks
                    return {0: (load_k,), 1: (load_q, rope_k_muls),
                            2: (rope_rest,)}
                return {-1: (load_k,), 0: (load_q,), 1: (rope_k_muls,),
                        2: (rope_rest,)}

            accs = {0: emit_load_boot()}

            for bh in range(NBH):
                state = {}
                hooks = make_hooks(bh, accs, state)
                for fn in hooks.get(-1, ()):
                    fn()
                emit_attention(bh, accs[bh], hooks)

    nc.compile()
    return nc


def _get_nc():
    if "nc" not in _CACHE:
        _CACHE["nc"] = _build()
    return _CACHE["nc"]


def _deint_T(x):
    # [N, S, D] -> de-interleave d (evens|odds) then transpose -> [N, D, S]
    return np.ascontiguousarray(
        np.concatenate([x[:, :, 0::2], x[:, :, 1::2]], axis=2)
        .transpose(0, 2, 1)).astype(np.float16)


def _fuse_eo(xT):
    # [N, D, S] (evens|odds on partitions) -> [N, 128, 2S] fused
    # (E|O) with E = (x0|x0), O = (x1|x1) duplicated across halves.
    E = np.concatenate([xT[:, 0:64], xT[:, 0:64]], axis=1)
    O = np.concatenate([xT[:, 64:128], xT[:, 64:128]], axis=1)
    return np.ascontiguousarray(np.concatenate([E, O], axis=2))


def _shard(q, k, v, freqs):
    q = np.asarray(q, dtype=np.float32)
    k = np.asarray(k, dtype=np.float32)
    v = np.asarray(v, dtype=np.float32)
    freqs = np.asarray(freqs, dtype=np.float32).reshape(S, D)

    def _ctile(t):
        # [D, S] fp16 -> chunk-tiled [4, D, 512] (each chunk contiguous)
        return np.ascontiguousarray(t.reshape(D, 4, 512).transpose(1, 0, 2))

    fT = _ctile(np.concatenate([freqs[:, 0::2], freqs[:, 1::2]], axis=1)
                .T.astype(np.float16))
    gT = _ctile(np.concatenate([-freqs[:, 1::2], freqs[:, 0::2]], axis=1)
                .T.astype(np.float16))
    in_maps = []
    for c in range(NCORES):
        h0 = c * HPC

        def bhslice(x):
            # [B, S, Hc, D] -> [B, Hc, S, D] -> [NBH, S, D]
            return np.ascontiguousarray(
                x[:, :, h0:h0 + HPC, :].transpose(0, 2, 1, 3)
            ).reshape(NBH, S, D)

        # v s-tiled: vt[bh, p, t*128+d] = v[bh, t*128+p, d]
        vt = np.ascontiguousarray(
            bhslice(v).reshape(NBH, NT, 128, D).transpose(0, 2, 1, 3)
        ).reshape(NBH, 128, S).astype(np.float16)

        qT = _deint_T(bhslice(q))
        kT = _deint_T(bhslice(k))
        qEO = _fuse_eo(qT)
        kEO = _fuse_eo(kT)
        # boot chunks: [128, 1024] per 512-col chunk = (E_c | O_c)
        def _bctile(xEO):
            # [128, 2S] -> [4, 128, 1024]
            E = xEO[0, :, 0:S].reshape(128, 4, 512)
            O = xEO[0, :, S:2 * S].reshape(128, 4, 512)
            return np.ascontiguousarray(
                np.concatenate([E, O], axis=2).transpose(1, 0, 2))

        in_maps.append({
            "qEO": qEO,
            "kEO": kEO,
            "qEOb": _bctile(qEO),
            "kEOb": _bctile(kEO),
            "v": vt,
            "freqsT": fT,
            "freqsG": gT,
        })
    return in_maps


def kernel(q, k, v, freqs):
    nc = _get_nc()
    from concourse.bass_utils import run_bass_kernel_spmd

    in_maps = _shard(q, k, v, freqs)
    res = run_bass_kernel_spmd(nc, in_maps, core_ids=list(range(NCORES)))

    out = np.empty((B, H, S, D), dtype=np.float32)
    for c in range(NCORES):
        h0 = c * HPC
        # out dram [NBH, 128, S] = unnormalized outT[d, q].
        # accs dram [NBH, QB, 128, 1536]: per q-block, three 512-col
        # segments of partial k-tile sums (qb0: only segment 0 valid);
        # sums[q] = sum over segments and partitions.
        oT = res.results[c]["out"].astype(np.float32)       # [NBH, D, S]
        ac = res.results[c]["accs"].astype(np.float32)      # [NBH,QB,128,1536]
        sums = ac[:, :, :, 0:512].sum(axis=2)               # [NBH, QB, 512]
        sums[:, 1:] += ac[:, 1:, :, 512:1024].sum(axis=2)
        sums[:, 1:] += ac[:, 1:, :, 1024:1536].sum(axis=2)
        sums = sums.reshape(NBH, S)
        out[:, h0:h0 + HPC] = (
            oT / sums[:, None, :]).transpose(0, 2, 1).reshape(B, HPC, S, D)
    return out


# revision 9
# speedup vs baseline: 1.0392x; 1.0392x over previous
"""Trainium2 Bass kernel: Wan-style interleaved RoPE on q/k + causal attention.

Full problem: q,k,v [B=2, S=2048, H=16, D=128] fp32, freqs [1, S, 1, D].
  rq = rope(q), rk = rope(k)
  out[b,h,q,d] = softmax_causal(rq @ rk^T / sqrt(D)) @ v      -> [B, H, S, D]

Sharding: heads split across 8 cores (2 heads/core); each core handles
4 independent (b, h) attention problems. Inputs are sliced on host, the
SPMD kernel runs on cores 0-7, outputs are concatenated on host.

Layout trick: scores = sum_d rq[d]*rk[d] are invariant under any shared
permutation of d, so q and k are shipped de-interleaved (evens then
odds), pre-transposed to [D', S], AND pre-duplicated: the vector
engines have no cross-partition path, so each shipped tensor is
[128, 2S] = (E | O) with E = (x0|x0), O = (x1|x1) stacked so that with
FF = (f0|f1), GG = (-f1|f0):
  rqT' = E*FF + O*GG    -- exactly interleaved RoPE in (evens|odds)
                           d-order.
One fused DMA loads (E|O) per tensor (same bytes as the four half-DMAs
it replaces -- the duplication is in DRAM -- but 4x fewer descriptors).

Everything is shipped and computed in fp16.  Attention per (b,h), per
q-block of 512: k-tiles are bin-packed into 512-col PSUM banks, three
banks per [128, 1536] score tile, diagonal tiles reordered j0,j1,j3,j2
so the packing has no holes; one exp(s*scale - 8) per batch (the
uniform bias cancels in the normalization and keeps exp in fp16 range).

Softmax sums and normalization are finished on the HOST: the kernel
merges each q-block's probT tiles with an in-place wide tree on DVE --
q-aligned full batches are added [1536]-wide at 2x perf mode into batch
0's tile, misaligned diagonal tiles are added at their q-offsets into
segment 0 -- and ships batch 0 raw ([128,1536]; [128,512] for qb0).
The host folds the 512-col segments, reduces the 128 partitions,
and divides the unnormalized output (outT evacuated fp32->fp16 by
ScalarE into out_full [d, q]) by the sums.

Engine balance: q RoPE (2 mul + add) and k's add on DVE; k's two muls
on GpSimd; exp + outT evac on ScalarE; out/acc DMAs issue from GpSimd,
loads from Sync.  Load DMAs are staggered (k at attention start, q
after qb0, v after qb2) and the next (b,h)'s RoPE is emitted in two
stages (k muls after qb1, the DVE ops after qb2 -- so the DVE add
gated on GpSimd never blocks the in-order vector queue) to smooth the
DMA/power bursts that otherwise stall DVE ops 10-20x.

Boot: (b,h)=0's q/k are shipped a second time chunk-tiled [4, 128,
1024] ((E|O) per 512-col chunk, contiguous in DRAM) so boot loads+RoPE
run chunk-by-chunk and qb0's matmuls start early; freqs/v issue from
the otherwise-idle Scalar queue.
"""

import math

import numpy as np

B, S, H, D = 2, 2048, 16, 128
NCORES = 8
HPC = H // NCORES          # heads per core
NBH = B * HPC              # (b, h) problems per core
NT = S // 128              # s-tiles
QB = S // 512              # q blocks of 512
SCALE = 1.0 / math.sqrt(D)
NEG = -1e30
EXPBIAS = 8.0              # uniform softmax shift; keeps exp in fp16 range
SCW = 1536                 # packed score-tile width (3 PSUM banks)

_CACHE = {}


def _plan(qb):
    """Pack this q-block's k-tiles into contiguous score batches.

    A matmul output must not cross a 512-col PSUM bank boundary, so tiles
    are bin-packed into 512-col banks (3 banks per [128, SCW] score
    tile).  The diagonal tiles (widths 512/384/256/128) are emitted in
    the order j0, j1, j3, j2 so banks fill exactly ([512], [384+128],
    [256]) with no holes: each batch's valid columns are contiguous from
    0 and one exp instruction covers them.  The first tile (tk=0, full
    width) stays first so its start=True matmul resets every PSUM cell
    of the PV accumulator.

    Returns (nk, batches); each batch is a list of (tk, off, lo, w).
    Tiles with off == 0 always land at 512-aligned lo (bank starts), so
    the leading off==0 run of every batch is q-aligned for wide adds.
    """
    nk = 4 * qb + 4
    order = list(range(4 * qb)) + [4 * qb, 4 * qb + 1, 4 * qb + 3, 4 * qb + 2]
    batches, cur = [], []
    bank, used = 0, 0
    for tk in order:
        j = tk - 4 * qb
        off = 128 * j if j > 0 else 0
        w = 512 - off
        if used + w > 512:
            bank, used = bank + 1, 0
        if bank == SCW // 512:
            batches.append(cur)
            cur, bank = [], 0
        cur.append((tk, off, bank * 512 + used, w))
        used += w
    batches.append(cur)
    return nk, batches


def _build():
    import concourse.mybir as mybir
    import concourse.tile as tile
    from concourse import bacc
    from concourse.masks import make_identity

    f32 = mybir.dt.float32
    f16 = mybir.dt.float16
    bf16 = mybir.dt.bfloat16
    Alu = mybir.AluOpType
    Act = mybir.ActivationFunctionType

    nc = bacc.Bacc("TRN2", target_bir_lowering=False, debug=False,
                   num_devices=NCORES)
    # steady-state loads: fused (E|O) [128, 2S] per (b,h)
    qd = nc.dram_tensor("qEO", [NBH, 128, 2 * S], f16, kind="ExternalInput")
    kd = nc.dram_tensor("kEO", [NBH, 128, 2 * S], f16, kind="ExternalInput")
    vd = nc.dram_tensor("v", [NBH, 128, S], f16, kind="ExternalInput")
    # boot copies of (b,h)=0's q/k, chunk-tiled [4, 128, 1024] ((E|O) per
    # 512-col chunk, contiguous): boot loads+RoPE run chunk-by-chunk.
    qbd = nc.dram_tensor("qEOb", [4, 128, 1024], f16, kind="ExternalInput")
    kbd = nc.dram_tensor("kEOb", [4, 128, 1024], f16, kind="ExternalInput")
    fd = nc.dram_tensor("freqsT", [4, D, 512], f16, kind="ExternalInput")
    gd = nc.dram_tensor("freqsG", [4, D, 512], f16, kind="ExternalInput")
    od = nc.dram_tensor("out", [NBH, 128, S], f16, kind="ExternalOutput")
    ad = nc.dram_tensor("accs", [NBH, QB, 128, SCW], f16,
                        kind="ExternalOutput")

    with tile.TileContext(nc) as tc:
        with (
            tc.tile_pool(name="const", bufs=1) as cpool,
            tc.tile_pool(name="io", bufs=2) as iopool,
            tc.tile_pool(name="rope", bufs=2) as rpool,
            tc.tile_pool(name="xt", bufs=2) as xtpool,
            tc.tile_pool(name="prob", bufs=8) as ppool,
            tc.tile_pool(name="outf", bufs=2) as opool,
            tc.tile_pool(name="sc_ps", bufs=2, space="PSUM") as sc_ps,
            tc.tile_pool(name="out_ps", bufs=2, space="PSUM") as out_ps,
        ):
            # ---- constants ----
            ident = cpool.tile([128, 128], f32, tag="ident")
            make_identity(nc, ident[:])
            # tri_bf[k, t] = 0 where k <= t (valid), NEG where k > t.
            tri_bf = cpool.tile([128, 128], bf16, tag="tri_bf")
            nc.gpsimd.memset(tri_bf[:], 0.0)
            nc.gpsimd.affine_select(
                out=tri_bf[:], in_=tri_bf[:],
                compare_op=Alu.is_ge, fill=NEG, base=0,
                pattern=[[1, 128]], channel_multiplier=-1,
            )
            ident_bf = cpool.tile([128, 128], bf16, tag="ident_bf")
            nc.vector.tensor_copy(ident_bf[:], ident[:])
            nbias = cpool.tile([128, 1], f32, tag="nbias")
            nc.vector.memset(nbias[:], -EXPBIAS)
            FF = cpool.tile([128, S], f16, tag="FF")
            GG = cpool.tile([128, S], f16, tag="GG")

            def rope_compute(xEO, xT_ap, mul_eng, add_eng, cs=slice(0, S)):
                n = cs.stop - cs.start
                xE = xEO[:, 0:n]
                xO = xEO[:, n:2 * n]
                mul_eng.tensor_mul(xE, xE, FF[:, cs])
                mul_eng.tensor_mul(xO, xO, GG[:, cs])
                add_eng.tensor_add(xT_ap, xE, xO)

            def emit_load_boot():
                """Chunked load+RoPE for (b,h)=0 from the chunk-tiled boot
                tensors: qb0's matmuls start as soon as chunk 0 lands.
                k's muls on GpSimd, q's on DVE (they run in parallel);
                freqs/v issue from the otherwise-idle Scalar queue."""
                qTc = [xtpool.tile([128, 512], f16, tag=f"bqT{c}",
                                   name=f"bqT{c}", bufs=1) for c in range(4)]
                kTc = [xtpool.tile([128, 512], f16, tag=f"bkT{c}",
                                   name=f"bkT{c}", bufs=1) for c in range(4)]
                for c in range(4):
                    cs = slice(c * 512, (c + 1) * 512)
                    nc.scalar.dma_start(FF[:, cs], fd.ap()[c])
                    nc.scalar.dma_start(GG[:, cs], gd.ap()[c])
                    kEO = rpool.tile([128, 1024], f16, tag="bkEO",
                                     name="bkEO")
                    nc.sync.dma_start(kEO[:], kbd.ap()[c])
                    rope_compute(kEO, kTc[c][:], nc.vector, nc.vector, cs)
                    qEO = rpool.tile([128, 1024], f16, tag="bqEO",
                                     name="bqEO")
                    nc.gpsimd.dma_start(qEO[:], qbd.ap()[c])
                    rope_compute(qEO, qTc[c][:], nc.vector, nc.vector, cs)
                    if c == 0:
                        v_mm = iopool.tile([128, S], f16, tag="v_mm",
                                           name="v_mm")
                        nc.scalar.dma_start(v_mm[:], vd.ap()[0])

                def kT_lhsT(tk):
                    return kTc[tk // 4][:, (tk % 4) * 128:(tk % 4 + 1) * 128]

                def qT_rhs(qb, off):
                    return qTc[qb][:, off:512]

                return (kT_lhsT, qT_rhs, v_mm)

            def emit_attention(bh, acc, hooks):
                kT_lhsT, qT_rhs, v_mm = acc
                last = bh == NBH - 1
                out_full = opool.tile([128, S], f16, tag="out_full",
                                      name="out_full")

                def phase_compute(qb):
                    nk, batches = _plan(qb)
                    last_tk = batches[-1][-1][0]
                    outs = out_full[:, qb * 512:(qb + 1) * 512]
                    outT = out_ps.tile([128, 512], f32, tag="outT",
                                       name="outT")
                    probts = []
                    na0 = sum(1 for t in batches[0] if t[1] == 0)
                    for batch in batches:
                        sc = sc_ps.tile([128, SCW], f32, tag="sc", name="sc")
                        for tk, off, lo, w in batch:
                            diag = tk >= 4 * qb
                            nc.tensor.matmul(
                                sc[:, lo:lo + w], kT_lhsT(tk),
                                qT_rhs(qb, off),
                                start=True, stop=not diag,
                            )
                            if diag:
                                nc.tensor.matmul(
                                    sc[:, lo:lo + 128],
                                    ident_bf[:], tri_bf[:],
                                    start=False, stop=True,
                                )
                        wtot = batch[-1][2] + batch[-1][3]
                        probt = ppool.tile([128, SCW], f16, tag="probt",
                                           name="probt")
                        nc.scalar.activation(
                            probt[:, 0:wtot], sc[:, 0:wtot],
                            Act.Exp, scale=SCALE, bias=nbias[:],
                        )
                        for tk, off, lo, w in batch:
                            nc.tensor.matmul(
                                outT[:, off:512],
                                v_mm[:, tk * 128:(tk + 1) * 128],
                                probt[:, lo:lo + w],
                                start=(tk == 0), stop=(tk == last_tk),
                            )
                        probts.append(probt)
                        # wide-tree merge: as soon as this batch's probt is
                        # final, add its q-aligned prefix (leading off==0
                        # tiles, always at 512-aligned lo) into batch 0's
                        # tile in place, [<=1536]-wide at DVE 2x rate.
                        if len(probts) > 1:
                            na = 0
                            for t in batch:
                                if t[1] != 0:
                                    break
                                na += 1
                            m = 512 * min(na, na0)
                            if m:
                                nc.vector.tensor_add(
                                    probts[0][:, 0:m], probts[0][:, 0:m],
                                    probt[:, 0:m])
                    # fold the misaligned diagonal tiles into batch 0's
                    # segment 0 at their q-offsets; the host folds the
                    # 512-col segments and reduces the 128 partitions.
                    p0 = probts[0]
                    for bi, batch in enumerate(batches):
                        seen_mis = False
                        for tk, off, lo, w in batch:
                            if off == 0 and not seen_mis:
                                continue
                            seen_mis = True
                            nc.vector.tensor_add(
                                p0[:, off:512], p0[:, off:512],
                                probts[bi][:, lo:lo + w])
                    nc.gpsimd.dma_start(
                        ad.ap()[bh, qb, :, 0:512 * na0],
                        p0[:, 0:512 * na0])
                    # evacuate the unnormalized PV accumulator fp32->fp16;
                    # host divides by the sums.  Alternate ScalarE/DVE per
                    # q-block to balance the two busiest engines.
                    if qb % 2 == 0:
                        nc.scalar.copy(outs, outT[:])
                    else:
                        nc.vector.tensor_copy(outs, outT[:])
                    if last:
                        nc.gpsimd.dma_start(
                            od.ap()[bh, :, qb * 512:(qb + 1) * 512], outs)

                # last (b,h): large q-blocks first so the post-PE tail
                # (DVE tree + DMA) of the final block is the smallest one.
                order = (3, 2, 1, 0) if last else range(QB)
                for qb in order:
                    phase_compute(qb)
                    for fn in hooks.get(qb, ()):
                        fn()
                if not last:
                    nc.gpsimd.dma_start(od.ap()[bh], out_full[:])

            def make_hooks(bh, accs, state):
                """Stagger next-(b,h) loads and RoPE across this (b,h)'s
                q-blocks to smooth DMA/power bursts:
                  qb0 end: k load     qb1 end: q load + k RoPE muls (GpSimd)
                  qb2 end: k RoPE add + q RoPE (DVE) + v load
                (k load issues at attention start, before qb0.)"""
                nxt = bh + 1
                if nxt >= NBH:
                    return {}

                def load_k():
                    kEO = rpool.tile([128, 2 * S], f16, tag="kEO",
                                     name="kEO")
                    nc.sync.dma_start(kEO[:], kd.ap()[nxt])
                    state["kEO"] = kEO

                def load_q():
                    qEO = rpool.tile([128, 2 * S], f16, tag="qEO",
                                     name="qEO")
                    nc.sync.dma_start(qEO[:], qd.ap()[nxt])
                    state["qEO"] = qEO

                def rope_k_muls():
                    # all RoPE runs on DVE: a Pool/GpSimd tensor_tensor
                    # running concurrently with ANY DVE op slows the DVE
                    # op 10-20x (measured), so GpSimd gets no TT work.
                    kT = xtpool.tile([128, S], f16, tag="kT", name="kT")
                    kEO = state["kEO"]
                    nc.vector.tensor_mul(kEO[:, 0:S], kEO[:, 0:S], FF[:])
                    nc.vector.tensor_mul(kEO[:, S:2 * S], kEO[:, S:2 * S],
                                         GG[:])
                    state["kT"] = kT

                def rope_rest():
                    kEO, qEO = state["kEO"], state["qEO"]
                    kT, qT = state["kT"], xtpool.tile([128, S], f16,
                                                      tag="qT", name="qT")
                    nc.vector.tensor_add(kT[:], kEO[:, 0:S], kEO[:, S:2 * S])
                    rope_compute(qEO, qT[:], nc.vector, nc.vector)
                    v_mm = iopool.tile([128, S], f16, tag="v_mm",
                                       name="v_mm")
                    nc.sync.dma_start(v_mm[:], vd.ap()[nxt])

                    def kT_lhsT(tk):
                        return kT[:, tk * 128:(tk + 1) * 128]

                    def qT_rhs(qb, off):
                        return qT[:, qb * 512 + off:(qb + 1) * 512]

                    accs[nxt] = (kT_lhsT, qT_rhs, v_mm)

                if bh == 0:
                    # don't compete with the latency-critical boot chunks
                    return {0: (load_k,), 1: (load_q, rope_k_muls),
                            2: (rope_rest,)}
                return {-1: (load_k,), 0: (load_q,), 1: (rope_k_muls,),
                        2: (rope_rest,)}

            accs = {0: emit_load_boot()}

            for bh in range(NBH):
                state = {}
                hooks = make_hooks(bh, accs, state)
                for fn in hooks.get(-1, ()):
                    fn()
                emit_attention(bh, accs[bh], hooks)

    nc.compile()
    return nc


def _get_nc():
    if "nc" not in _CACHE:
        _CACHE["nc"] = _build()
    return _CACHE["nc"]


def _deint_T(x):
    # [N, S, D] -> de-interleave d (evens|odds) then transpose -> [N, D, S]
    return np.ascontiguousarray(
        np.concatenate([x[:, :, 0::2], x[:, :, 1::2]], axis=2)
        .transpose(0, 2, 1)).astype(np.float16)


def _fuse_eo(xT):
    # [N, D, S] (evens|odds on partitions) -> [N, 128, 2S] fused
    # (E|O) with E = (x0|x0), O = (x1|x1) duplicated across halves.
    E = np.concatenate([xT[:, 0:64], xT[:, 0:64]], axis=1)
    O = np.concatenate([xT[:, 64:128], xT[:, 64:128]], axis=1)
    return np.ascontiguousarray(np.concatenate([E, O], axis=2))


def _shard(q, k, v, freqs):
    q = np.asarray(q, dtype=np.float32)
    k = np.asarray(k, dtype=np.float32)
    v = np.asarray(v, dtype=np.float32)
    freqs = np.asarray(freqs, dtype=np.float32).reshape(S, D)

    def _ctile(t):
        # [D, S] fp16 -> chunk-tiled [4, D, 512] (each chunk contiguous)
        return np.ascontiguousarray(t.reshape(D, 4, 512).transpose(1, 0, 2))

    fT = _ctile(np.concatenate([freqs[:, 0::2], freqs[:, 1::2]], axis=1)
                .T.astype(np.float16))
    gT = _ctile(np.concatenate([-freqs[:, 1::2], freqs[:, 0::2]], axis=1)
                .T.astype(np.float16))
    in_maps = []
    for c in range(NCORES):
        h0 = c * HPC

        def bhslice(x):
            # [B, S, Hc, D] -> [B, Hc, S, D] -> [NBH, S, D]
            return np.ascontiguousarray(
                x[:, :, h0:h0 + HPC, :].transpose(0, 2, 1, 3)
            ).reshape(NBH, S, D)

        # v s-tiled: vt[bh, p, t*128+d] = v[bh, t*128+p, d]
        vt = np.ascontiguousarray(
            bhslice(v).reshape(NBH, NT, 128, D).transpose(0, 2, 1, 3)
        ).reshape(NBH, 128, S).astype(np.float16)

        qT = _deint_T(bhslice(q))
        kT = _deint_T(bhslice(k))
        qEO = _fuse_eo(qT)
        kEO = _fuse_eo(kT)
        # boot chunks: [128, 1024] per 512-col chunk = (E_c | O_c)
        def _bctile(xEO):
            # [128, 2S] -> [4, 128, 1024]
            E = xEO[0, :, 0:S].reshape(128, 4, 512)
            O = xEO[0, :, S:2 * S].reshape(128, 4, 512)
            return np.ascontiguousarray(
                np.concatenate([E, O], axis=2).transpose(1, 0, 2))

        in_maps.append({
            "qEO": qEO,
            "kEO": kEO,
            "qEOb": _bctile(qEO),
            "kEOb": _bctile(kEO),
            "v": vt,
            "freqsT": fT,
            "freqsG": gT,
        })
    return in_maps


def kernel(q, k, v, freqs):
    nc = _get_nc()
    from concourse.bass_utils import run_bass_kernel_spmd

    in_maps = _shard(q, k, v, freqs)
    res = run_bass_kernel_spmd(nc, in_maps, core_ids=list(range(NCORES)))

    out = np.empty((B, H, S, D), dtype=np.float32)
    for c in range(NCORES):
        h0 = c * HPC
        # out dram [NBH, 128, S] = unnormalized outT[d, q].
        # accs dram [NBH, QB, 128, 1536]: per q-block, three 512-col
        # segments of partial k-tile sums (qb0: only segment 0 valid);
        # sums[q] = sum over segments and partitions.
        oT = res.results[c]["out"].astype(np.float32)       # [NBH, D, S]
        ac = res.results[c]["accs"].astype(np.float32)      # [NBH,QB,128,1536]
        sums = ac[:, :, :, 0:512].sum(axis=2)               # [NBH, QB, 512]
        sums[:, 1:] += ac[:, 1:, :, 512:1024].sum(axis=2)
        sums[:, 1:] += ac[:, 1:, :, 1024:1536].sum(axis=2)
        sums = sums.reshape(NBH, S)
        out[:, h0:h0 + HPC] = (
            oT / sums[:, None, :]).transpose(0, 2, 1).reshape(B, HPC, S, D)
    return out


# revision 16
# speedup vs baseline: 1.2309x; 1.1845x over previous
"""Trainium2 Bass kernel: Wan-style interleaved RoPE on q/k + causal attention.

Full problem: q,k,v [B=2, S=2048, H=16, D=128] fp32, freqs [1, S, 1, D].
  rq = rope(q), rk = rope(k)
  out[b,h,q,d] = softmax_causal(rq @ rk^T / sqrt(D)) @ v      -> [B, H, S, D]

Sharding: heads split across 8 cores (2 heads/core); each core handles
4 independent (b, h) attention problems. Inputs are sliced on host, the
SPMD kernel runs on cores 0-7, outputs are concatenated on host.

Layout trick: scores = sum_d rq[d]*rk[d] are invariant under any shared
permutation of d, so q and k are shipped de-interleaved (evens then
odds), pre-transposed to [D', S], AND pre-duplicated: the vector
engines have no cross-partition path, so each shipped tensor is
[128, 2S] = (E | O) with E = (x0|x0), O = (x1|x1) stacked so that with
FF = (f0|f1), GG = (-f1|f0):
  rqT' = E*FF + O*GG    -- exactly interleaved RoPE in (evens|odds)
                           d-order.
One fused DMA loads (E|O) per tensor (same bytes as the four half-DMAs
it replaces -- the duplication is in DRAM -- but 4x fewer descriptors).

Everything is shipped and computed in fp16.  Attention per (b,h), per
q-block of 512: k-tiles are bin-packed into 512-col PSUM banks, three
banks per [128, 1536] score tile, diagonal tiles reordered j0,j1,j3,j2
so the packing has no holes; one exp(s*scale - 8) per batch (the
uniform bias cancels in the normalization and keeps exp in fp16 range).

Softmax sums and normalization are finished on the HOST: the kernel
merges each q-block's probT tiles with an in-place wide tree on DVE --
q-aligned full batches are added [1536]-wide at 2x perf mode into batch
0's tile, misaligned diagonal tiles are added at their q-offsets into
segment 0 -- and ships batch 0 raw ([128,1536]; [128,512] for qb0).
The host folds the 512-col segments, reduces the 128 partitions,
and divides the unnormalized output (outT evacuated fp32->fp16 by
ScalarE into out_full [d, q]) by the sums.

Engine balance: q RoPE (2 mul + add) and k's add on DVE; k's two muls
on GpSimd; exp + outT evac on ScalarE; out/acc DMAs issue from GpSimd,
loads from Sync.  Load DMAs are staggered (k at attention start, q
after qb0, v after qb2) and the next (b,h)'s RoPE is emitted in two
stages (k muls after qb1, the DVE ops after qb2 -- so the DVE add
gated on GpSimd never blocks the in-order vector queue) to smooth the
DMA/power bursts that otherwise stall DVE ops 10-20x.

Boot: (b,h)=0's q/k are shipped a second time chunk-tiled [4, 128,
1024] ((E|O) per 512-col chunk, contiguous in DRAM) so boot loads+RoPE
run chunk-by-chunk and qb0's matmuls start early; freqs/v issue from
the otherwise-idle Scalar queue.
"""

import math

import numpy as np

B, S, H, D = 2, 2048, 16, 128
NCORES = 8
HPC = H // NCORES          # heads per core
NBH = B * HPC              # (b, h) problems per core
NT = S // 128              # s-tiles
QB = S // 512              # q blocks of 512
SCALE = 1.0 / math.sqrt(D)
NEG = -1e30
EXPBIAS = 8.0              # uniform softmax shift; keeps exp in fp16 range
SCW = 1536                 # packed score-tile width (3 PSUM banks)

_CACHE = {}


def _plan(qb):
    """Pack this q-block's k-tiles into contiguous score batches.

    A matmul output must not cross a 512-col PSUM bank boundary, so tiles
    are bin-packed into 512-col banks (3 banks per [128, SCW] score
    tile).  The diagonal tiles (widths 512/384/256/128) are emitted in
    the order j0, j1, j3, j2 so banks fill exactly ([512], [384+128],
    [256]) with no holes: each batch's valid columns are contiguous from
    0 and one exp instruction covers them.  The first tile (tk=0, full
    width) stays first so its start=True matmul resets every PSUM cell
    of the PV accumulator.

    Returns (nk, batches); each batch is a list of (tk, off, lo, w).
    Tiles with off == 0 always land at 512-aligned lo (bank starts), so
    the leading off==0 run of every batch is q-aligned for wide adds.
    """
    nk = 4 * qb + 4
    order = list(range(4 * qb)) + [4 * qb, 4 * qb + 1, 4 * qb + 3, 4 * qb + 2]
    batches, cur = [], []
    bank, used = 0, 0
    for tk in order:
        j = tk - 4 * qb
        off = 128 * j if j > 0 else 0
        w = 512 - off
        if used + w > 512:
            bank, used = bank + 1, 0
        if bank == SCW // 512:
            batches.append(cur)
            cur, bank = [], 0
        cur.append((tk, off, bank * 512 + used, w))
        used += w
    batches.append(cur)
    return nk, batches


def _build():
    import concourse.mybir as mybir
    import concourse.tile as tile
    from concourse import bacc
    from concourse.masks import make_identity

    f32 = mybir.dt.float32
    f16 = mybir.dt.float16
    bf16 = mybir.dt.bfloat16
    Alu = mybir.AluOpType
    Act = mybir.ActivationFunctionType

    nc = bacc.Bacc("TRN2", target_bir_lowering=False, debug=False,
                   num_devices=NCORES)
    # steady-state loads: fused (E|O) [128, 2S] per (b,h)
    qd = nc.dram_tensor("qEO", [NBH, 128, 2 * S], f16, kind="ExternalInput")
    kd = nc.dram_tensor("kEO", [NBH, 128, 2 * S], f16, kind="ExternalInput")
    vd = nc.dram_tensor("v", [NBH, 128, S], f16, kind="ExternalInput")
    # boot copies of (b,h)=0's q/k, chunk-tiled [4, 128, 1024] ((E|O) per
    # 512-col chunk, contiguous): boot loads+RoPE run chunk-by-chunk.
    qbd = nc.dram_tensor("qEOb", [4, 128, 1024], f16, kind="ExternalInput")
    kbd = nc.dram_tensor("kEOb", [4, 128, 1024], f16, kind="ExternalInput")
    fd = nc.dram_tensor("freqsT", [D, S], f16, kind="ExternalInput")
    gd = nc.dram_tensor("freqsG", [D, S], f16, kind="ExternalInput")
    od = nc.dram_tensor("out", [NBH, 128, S], f16, kind="ExternalOutput")
    ad = nc.dram_tensor("accs", [NBH, QB, 128, SCW], f16,
                        kind="ExternalOutput")

    with tile.TileContext(nc) as tc:
        with (
            tc.tile_pool(name="const", bufs=1) as cpool,
            tc.tile_pool(name="io", bufs=2) as iopool,
            tc.tile_pool(name="rope", bufs=2) as rpool,
            tc.tile_pool(name="xt", bufs=2) as xtpool,
            tc.tile_pool(name="prob", bufs=8) as ppool,
            tc.tile_pool(name="outf", bufs=2) as opool,
            tc.tile_pool(name="sc_ps", bufs=2, space="PSUM") as sc_ps,
            tc.tile_pool(name="out_ps", bufs=2, space="PSUM") as out_ps,
        ):
            # ---- constants (tiles only; ops are emitted mid-boot so the
            # latency-critical chunk-0 loads+RoPE go first on each queue) --
            tri_bf = cpool.tile([128, 128], bf16, tag="tri_bf")
            ident_bf = cpool.tile([128, 128], bf16, tag="ident_bf")
            nbias = cpool.tile([128, 1], f32, tag="nbias")
            FF = cpool.tile([128, S], f16, tag="FF")
            GG = cpool.tile([128, S], f16, tag="GG")

            def emit_consts():
                make_identity(nc, ident_bf[:])
                # tri_bf[k, t] = 0 where k <= t (valid), NEG where k > t.
                nc.gpsimd.memset(tri_bf[:], 0.0)
                nc.gpsimd.affine_select(
                    out=tri_bf[:], in_=tri_bf[:],
                    compare_op=Alu.is_ge, fill=NEG, base=0,
                    pattern=[[1, 128]], channel_multiplier=-1,
                )
                nc.vector.memset(nbias[:], -EXPBIAS)

            def rope_compute(xEO, xT_ap, mul_eng, add_eng, cs=slice(0, S)):
                n = cs.stop - cs.start
                xE = xEO[:, 0:n]
                xO = xEO[:, n:2 * n]
                mul_eng.tensor_mul(xE, xE, FF[:, cs])
                mul_eng.tensor_mul(xO, xO, GG[:, cs])
                add_eng.tensor_add(xT_ap, xE, xO)

            def emit_load_boot():
                """Chunked load+RoPE for (b,h)=0 from the chunk-tiled boot
                tensors: qb0's matmuls start as soon as chunk 0 lands.
                Freqs load as two full-tensor DMAs (Scalar), v from the
                Tensor queue (own DMA ring, lands early), k chunks from
                Sync, q chunks from GpSimd; all RoPE on DVE."""
                nc.scalar.dma_start(FF[:], fd.ap()[:, :])
                nc.scalar.dma_start(GG[:], gd.ap()[:, :])
                v_mm = iopool.tile([128, S], f16, tag="v_mm", name="v_mm")
                qTc = [xtpool.tile([128, 512], f16, tag=f"bqT{c}",
                                   name=f"bqT{c}", bufs=1) for c in range(4)]
                kTc = [xtpool.tile([128, 512], f16, tag=f"bkT{c}",
                                   name=f"bkT{c}", bufs=1) for c in range(4)]
                for c in range(4):
                    cs = slice(c * 512, (c + 1) * 512)
                    kEO = rpool.tile([128, 1024], f16, tag="bkEO",
                                     name="bkEO")
                    nc.sync.dma_start(kEO[:], kbd.ap()[c])
                    qEO = rpool.tile([128, 1024], f16, tag="bqEO",
                                     name="bqEO")
                    nc.gpsimd.dma_start(qEO[:], qbd.ap()[c])
                    rope_compute(kEO, kTc[c][:], nc.vector, nc.vector, cs)
                    rope_compute(qEO, qTc[c][:], nc.vector, nc.vector, cs)
                    if c == 0:
                        emit_consts()
                    if c == 1:
                        # v rides the Sync ring behind the first two k
                        # chunks; needed by the first PV (~after exp qb0).
                        nc.sync.dma_start(v_mm[:], vd.ap()[0])

                def kT_lhsT(tk):
                    return kTc[tk // 4][:, (tk % 4) * 128:(tk % 4 + 1) * 128]

                def qT_rhs(qb, off):
                    return qTc[qb][:, off:512]

                return (kT_lhsT, qT_rhs, v_mm)

            def emit_attention(bh, acc, hooks):
                kT_lhsT, qT_rhs, v_mm = acc
                last = bh == NBH - 1
                out_full = opool.tile([128, S], f16, tag="out_full",
                                      name="out_full")

                def phase_compute(qb):
                    nk, batches = _plan(qb)
                    last_tk = batches[-1][-1][0]
                    outs = out_full[:, qb * 512:(qb + 1) * 512]
                    outT = out_ps.tile([128, 512], f32, tag="outT",
                                       name="outT")
                    probts = []
                    na0 = sum(1 for t in batches[0] if t[1] == 0)
                    for batch in batches:
                        sc = sc_ps.tile([128, SCW], f32, tag="sc", name="sc")
                        for tk, off, lo, w in batch:
                            diag = tk >= 4 * qb
                            nc.tensor.matmul(
                                sc[:, lo:lo + w], kT_lhsT(tk),
                                qT_rhs(qb, off),
                                start=True, stop=not diag,
                            )
                            if diag:
                                nc.tensor.matmul(
                                    sc[:, lo:lo + 128],
                                    ident_bf[:], tri_bf[:],
                                    start=False, stop=True,
                                )
                        wtot = batch[-1][2] + batch[-1][3]
                        probt = ppool.tile([128, SCW], f16, tag="probt",
                                           name="probt")
                        nc.scalar.activation(
                            probt[:, 0:wtot], sc[:, 0:wtot],
                            Act.Exp, scale=SCALE, bias=nbias[:],
                        )
                        for tk, off, lo, w in batch:
                            nc.tensor.matmul(
                                outT[:, off:512],
                                v_mm[:, tk * 128:(tk + 1) * 128],
                                probt[:, lo:lo + w],
                                start=(tk == 0), stop=(tk == last_tk),
                            )
                        probts.append(probt)
                        # wide-tree merge: as soon as this batch's probt is
                        # final, add its q-aligned prefix (leading off==0
                        # tiles, always at 512-aligned lo) into batch 0's
                        # tile in place, [<=1536]-wide at DVE 2x rate.
                        if len(probts) > 1:
                            na = 0
                            for t in batch:
                                if t[1] != 0:
                                    break
                                na += 1
                            m = 512 * min(na, na0)
                            if m:
                                nc.vector.tensor_add(
                                    probts[0][:, 0:m], probts[0][:, 0:m],
                                    probt[:, 0:m])
                    # fold the misaligned diagonal tiles into batch 0's
                    # segment 0 at their q-offsets; the host folds the
                    # 512-col segments and reduces the 128 partitions.
                    p0 = probts[0]
                    for bi, batch in enumerate(batches):
                        seen_mis = False
                        for tk, off, lo, w in batch:
                            if off == 0 and not seen_mis:
                                continue
                            seen_mis = True
                            nc.vector.tensor_add(
                                p0[:, off:512], p0[:, off:512],
                                probts[bi][:, lo:lo + w])
                    nc.gpsimd.dma_start(
                        ad.ap()[bh, qb, :, 0:512 * na0],
                        p0[:, 0:512 * na0])
                    # evacuate the unnormalized PV accumulator fp32->fp16;
                    # host divides by the sums.  ScalarE (exp-bound) takes
                    # only one per (b,h); DVE the rest.
                    if qb == 0:
                        nc.scalar.copy(outs, outT[:])
                    else:
                        nc.vector.tensor_copy(outs, outT[:])
                    if last:
                        nc.gpsimd.dma_start(
                            od.ap()[bh, :, qb * 512:(qb + 1) * 512], outs)

                # last (b,h): large q-blocks first so the post-PE tail
                # (DVE tree + DMA) of the final block is the smallest one.
                order = (3, 2, 1, 0) if last else range(QB)
                for qb in order:
                    phase_compute(qb)
                    for fn in hooks.get(qb, ()):
                        fn()
                if not last:
                    nc.gpsimd.dma_start(od.ap()[bh], out_full[:])

            def make_hooks(bh, accs, state):
                """Stagger next-(b,h) loads and RoPE across this (b,h)'s
                q-blocks to smooth DMA/power bursts:
                  qb0 end: k load     qb1 end: q load + k RoPE muls (GpSimd)
                  qb2 end: k RoPE add + q RoPE (DVE) + v load
                (k load issues at attention start, before qb0.)"""
                nxt = bh + 1
                if nxt >= NBH:
                    return {}

                def load_k():
                    kEO = rpool.tile([128, 2 * S], f16, tag="kEO",
                                     name="kEO")
                    nc.sync.dma_start(kEO[:], kd.ap()[nxt])
                    state["kEO"] = kEO

                def load_q():
                    qEO = rpool.tile([128, 2 * S], f16, tag="qEO",
                                     name="qEO")
                    nc.sync.dma_start(qEO[:], qd.ap()[nxt])
                    state["qEO"] = qEO

                def rope_k_muls():
                    # all RoPE runs on DVE: a Pool/GpSimd tensor_tensor
                    # running concurrently with ANY DVE op slows the DVE
                    # op 10-20x (measured), so GpSimd gets no TT work.
                    kT = xtpool.tile([128, S], f16, tag="kT", name="kT")
                    kEO = state["kEO"]
                    nc.vector.tensor_mul(kEO[:, 0:S], kEO[:, 0:S], FF[:])
                    nc.vector.tensor_mul(kEO[:, S:2 * S], kEO[:, S:2 * S],
                                         GG[:])
                    state["kT"] = kT

                def rope_rest():
                    kEO, qEO = state["kEO"], state["qEO"]
                    kT, qT = state["kT"], xtpool.tile([128, S], f16,
                                                      tag="qT", name="qT")
                    nc.vector.tensor_add(kT[:], kEO[:, 0:S], kEO[:, S:2 * S])
                    rope_compute(qEO, qT[:], nc.vector, nc.vector)
                    v_mm = iopool.tile([128, S], f16, tag="v_mm",
                                       name="v_mm")
                    nc.sync.dma_start(v_mm[:], vd.ap()[nxt])

                    def kT_lhsT(tk):
                        return kT[:, tk * 128:(tk + 1) * 128]

                    def qT_rhs(qb, off):
                        return qT[:, qb * 512 + off:(qb + 1) * 512]

                    accs[nxt] = (kT_lhsT, qT_rhs, v_mm)

                if bh == 0:
                    # don't compete with the latency-critical boot chunks
                    return {0: (load_k,), 1: (load_q, rope_k_muls),
                            2: (rope_rest,)}
                return {-1: (load_k,), 0: (load_q,), 1: (rope_k_muls,),
                        2: (rope_rest,)}

            accs = {0: emit_load_boot()}

            for bh in range(NBH):
                state = {}
                hooks = make_hooks(bh, accs, state)
                for fn in hooks.get(-1, ()):
                    fn()
                emit_attention(bh, accs[bh], hooks)

    nc.compile()
    return nc


def _get_nc():
    if "nc" not in _CACHE:
        _CACHE["nc"] = _build()
    return _CACHE["nc"]


def _deint_T(x):
    # [N, S, D] -> de-interleave d (evens|odds) then transpose -> [N, D, S]
    return np.ascontiguousarray(
        np.concatenate([x[:, :, 0::2], x[:, :, 1::2]], axis=2)
        .transpose(0, 2, 1)).astype(np.float16)


def _fuse_eo(xT):
    # [N, D, S] (evens|odds on partitions) -> [N, 128, 2S] fused
    # (E|O) with E = (x0|x0), O = (x1|x1) duplicated across halves.
    E = np.concatenate([xT[:, 0:64], xT[:, 0:64]], axis=1)
    O = np.concatenate([xT[:, 64:128], xT[:, 64:128]], axis=1)
    return np.ascontiguousarray(np.concatenate([E, O], axis=2))


def _shard(q, k, v, freqs):
    q = np.asarray(q, dtype=np.float32)
    k = np.asarray(k, dtype=np.float32)
    v = np.asarray(v, dtype=np.float32)
    freqs = np.asarray(freqs, dtype=np.float32).reshape(S, D)

    fT = np.ascontiguousarray(
        np.concatenate([freqs[:, 0::2], freqs[:, 1::2]], axis=1)
        .T.astype(np.float16))
    gT = np.ascontiguousarray(
        np.concatenate([-freqs[:, 1::2], freqs[:, 0::2]], axis=1)
        .T.astype(np.float16))
    in_maps = []
    for c in range(NCORES):
        h0 = c * HPC

        def bhslice(x):
            # [B, S, Hc, D] -> [B, Hc, S, D] -> [NBH, S, D]
            return np.ascontiguousarray(
                x[:, :, h0:h0 + HPC, :].transpose(0, 2, 1, 3)
            ).reshape(NBH, S, D)

        # v s-tiled: vt[bh, p, t*128+d] = v[bh, t*128+p, d]
        vt = np.ascontiguousarray(
            bhslice(v).reshape(NBH, NT, 128, D).transpose(0, 2, 1, 3)
        ).reshape(NBH, 128, S).astype(np.float16)

        qT = _deint_T(bhslice(q))
        kT = _deint_T(bhslice(k))
        qEO = _fuse_eo(qT)
        kEO = _fuse_eo(kT)
        # boot chunks: [128, 1024] per 512-col chunk = (E_c | O_c)
        def _bctile(xEO):
            # [128, 2S] -> [4, 128, 1024]
            E = xEO[0, :, 0:S].reshape(128, 4, 512)
            O = xEO[0, :, S:2 * S].reshape(128, 4, 512)
            return np.ascontiguousarray(
                np.concatenate([E, O], axis=2).transpose(1, 0, 2))

        in_maps.append({
            "qEO": qEO,
            "kEO": kEO,
            "qEOb": _bctile(qEO),
            "kEOb": _bctile(kEO),
            "v": vt,
            "freqsT": fT,
            "freqsG": gT,
        })
    return in_maps


def kernel(q, k, v, freqs):
    nc = _get_nc()
    from concourse.bass_utils import run_bass_kernel_spmd

    in_maps = _shard(q, k, v, freqs)
    res = run_bass_kernel_spmd(nc, in_maps, core_ids=list(range(NCORES)))

    out = np.empty((B, H, S, D), dtype=np.float32)
    for c in range(NCORES):
        h0 = c * HPC
        # out dram [NBH, 128, S] = unnormalized outT[d, q].
        # accs dram [NBH, QB, 128, 1536]: per q-block, three 512-col
        # segments of partial k-tile sums (qb0: only segment 0 valid);
        # sums[q] = sum over segments and partitions.
        oT = res.results[c]["out"].astype(np.float32)       # [NBH, D, S]
        ac = res.results[c]["accs"].astype(np.float32)      # [NBH,QB,128,1536]
        sums = ac[:, :, :, 0:512].sum(axis=2)               # [NBH, QB, 512]
        sums[:, 1:] += ac[:, 1:, :, 512:1024].sum(axis=2)
        sums[:, 1:] += ac[:, 1:, :, 1024:1536].sum(axis=2)
        sums = sums.reshape(NBH, S)
        out[:, h0:h0 + HPC] = (
            oT / sums[:, None, :]).transpose(0, 2, 1).reshape(B, HPC, S, D)
    return out


# revision 17
# speedup vs baseline: 1.2699x; 1.0317x over previous
"""Trainium2 Bass kernel: Wan-style interleaved RoPE on q/k + causal attention.

Full problem: q,k,v [B=2, S=2048, H=16, D=128] fp32, freqs [1, S, 1, D].
  rq = rope(q), rk = rope(k)
  out[b,h,q,d] = softmax_causal(rq @ rk^T / sqrt(D)) @ v      -> [B, H, S, D]

Sharding: heads split across 8 cores (2 heads/core); each core handles
4 independent (b, h) attention problems. Inputs are sliced on host, the
SPMD kernel runs on cores 0-7, outputs are concatenated on host.

Layout trick: scores = sum_d rq[d]*rk[d] are invariant under any shared
permutation of d, so q and k are shipped de-interleaved (evens then
odds), pre-transposed to [D', S], AND pre-duplicated: the vector
engines have no cross-partition path, so each shipped tensor is
[128, 2S] = (E | O) with E = (x0|x0), O = (x1|x1) stacked so that with
FF = (f0|f1), GG = (-f1|f0):
  rqT' = E*FF + O*GG    -- exactly interleaved RoPE in (evens|odds)
                           d-order.
One fused DMA loads (E|O) per tensor (same bytes as the four half-DMAs
it replaces -- the duplication is in DRAM -- but 4x fewer descriptors).

Everything is shipped and computed in fp16.  Attention per (b,h), per
q-block of 512: k-tiles are bin-packed into 512-col PSUM banks, three
banks per [128, 1536] score tile, diagonal tiles reordered j0,j1,j3,j2
so the packing has no holes; one exp(s*scale - 8) per batch (the
uniform bias cancels in the normalization and keeps exp in fp16 range).

Softmax sums and normalization are finished on the HOST: the kernel
merges each q-block's probT tiles with an in-place wide tree on DVE --
q-aligned full batches are added [1536]-wide at 2x perf mode into batch
0's tile, misaligned diagonal tiles are added at their q-offsets into
segment 0 -- and ships batch 0 raw ([128,1536]; [128,512] for qb0).
The host folds the 512-col segments, reduces the 128 partitions,
and divides the unnormalized output (outT evacuated fp32->fp16 by
ScalarE into out_full [d, q]) by the sums.

Engine balance: q RoPE (2 mul + add) and k's add on DVE; k's two muls
on GpSimd; exp + outT evac on ScalarE; out/acc DMAs issue from GpSimd,
loads from Sync.  Load DMAs are staggered (k at attention start, q
after qb0, v after qb2) and the next (b,h)'s RoPE is emitted in two
stages (k muls after qb1, the DVE ops after qb2 -- so the DVE add
gated on GpSimd never blocks the in-order vector queue) to smooth the
DMA/power bursts that otherwise stall DVE ops 10-20x.

Boot: (b,h)=0's q/k are shipped a second time chunk-tiled [4, 128,
1024] ((E|O) per 512-col chunk, contiguous in DRAM) so boot loads+RoPE
run chunk-by-chunk and qb0's matmuls start early; freqs/v issue from
the otherwise-idle Scalar queue.
"""

import math

import numpy as np

B, S, H, D = 2, 2048, 16, 128
NCORES = 8
HPC = H // NCORES          # heads per core
NBH = B * HPC              # (b, h) problems per core
NT = S // 128              # s-tiles
QB = S // 512              # q blocks of 512
SCALE = 1.0 / math.sqrt(D)
NEG = -1e30
EXPBIAS = 8.0              # uniform softmax shift; keeps exp in fp16 range
SCW = 1536                 # packed score-tile width (3 PSUM banks)

_CACHE = {}


def _plan(qb):
    """Pack this q-block's k-tiles into contiguous score batches.

    A matmul output must not cross a 512-col PSUM bank boundary, so tiles
    are bin-packed into 512-col banks (3 banks per [128, SCW] score
    tile).  The diagonal tiles (widths 512/384/256/128) are emitted in
    the order j0, j1, j3, j2 so banks fill exactly ([512], [384+128],
    [256]) with no holes: each batch's valid columns are contiguous from
    0 and one exp instruction covers them.  The first tile (tk=0, full
    width) stays first so its start=True matmul resets every PSUM cell
    of the PV accumulator.

    Returns (nk, batches); each batch is a list of (tk, off, lo, w).
    Tiles with off == 0 always land at 512-aligned lo (bank starts), so
    the leading off==0 run of every batch is q-aligned for wide adds.
    """
    nk = 4 * qb + 4
    order = list(range(4 * qb)) + [4 * qb, 4 * qb + 1, 4 * qb + 3, 4 * qb + 2]
    batches, cur = [], []
    bank, used = 0, 0
    for tk in order:
        j = tk - 4 * qb
        off = 128 * j if j > 0 else 0
        w = 512 - off
        if used + w > 512:
            bank, used = bank + 1, 0
        if bank == SCW // 512:
            batches.append(cur)
            cur, bank = [], 0
        cur.append((tk, off, bank * 512 + used, w))
        used += w
    batches.append(cur)
    return nk, batches


def _build():
    import concourse.mybir as mybir
    import concourse.tile as tile
    from concourse import bacc
    from concourse.masks import make_identity

    f32 = mybir.dt.float32
    f16 = mybir.dt.float16
    bf16 = mybir.dt.bfloat16
    Alu = mybir.AluOpType
    Act = mybir.ActivationFunctionType

    nc = bacc.Bacc("TRN2", target_bir_lowering=False, debug=False,
                   num_devices=NCORES)
    # steady-state loads: fused (E|O) [128, 2S] per (b,h)
    qd = nc.dram_tensor("qEO", [NBH, 128, 2 * S], f16, kind="ExternalInput")
    kd = nc.dram_tensor("kEO", [NBH, 128, 2 * S], f16, kind="ExternalInput")
    vd = nc.dram_tensor("v", [NBH, 128, S], f16, kind="ExternalInput")
    # boot copies of (b,h)=0's q/k, chunk-tiled [4, 128, 1024] ((E|O) per
    # 512-col chunk, contiguous): boot loads+RoPE run chunk-by-chunk.
    qbd = nc.dram_tensor("qEOb", [4, 128, 1024], f16, kind="ExternalInput")
    kbd = nc.dram_tensor("kEOb", [4, 128, 1024], f16, kind="ExternalInput")
    fd = nc.dram_tensor("freqsT", [D, S], f16, kind="ExternalInput")
    gd = nc.dram_tensor("freqsG", [D, S], f16, kind="ExternalInput")
    od = nc.dram_tensor("out", [NBH, 128, S], f16, kind="ExternalOutput")
    ad = nc.dram_tensor("accs", [NBH, QB, 128, SCW], f16,
                        kind="ExternalOutput")

    with tile.TileContext(nc) as tc:
        with (
            tc.tile_pool(name="const", bufs=1) as cpool,
            tc.tile_pool(name="io", bufs=2) as iopool,
            tc.tile_pool(name="rope", bufs=2) as rpool,
            tc.tile_pool(name="xt", bufs=2) as xtpool,
            tc.tile_pool(name="prob", bufs=8) as ppool,
            tc.tile_pool(name="outf", bufs=2) as opool,
            tc.tile_pool(name="sc_ps", bufs=2, space="PSUM") as sc_ps,
            tc.tile_pool(name="out_ps", bufs=2, space="PSUM") as out_ps,
        ):
            # ---- constants (tiles only; ops are emitted mid-boot so the
            # latency-critical chunk-0 loads+RoPE go first on each queue) --
            tri_bf = cpool.tile([128, 128], bf16, tag="tri_bf")
            ident_bf = cpool.tile([128, 128], bf16, tag="ident_bf")
            nbias = cpool.tile([128, 1], f32, tag="nbias")
            FF = cpool.tile([128, S], f16, tag="FF")
            GG = cpool.tile([128, S], f16, tag="GG")

            def emit_consts():
                make_identity(nc, ident_bf[:])
                # tri_bf[k, t] = 0 where k <= t (valid), NEG where k > t.
                nc.gpsimd.memset(tri_bf[:], 0.0)
                nc.gpsimd.affine_select(
                    out=tri_bf[:], in_=tri_bf[:],
                    compare_op=Alu.is_ge, fill=NEG, base=0,
                    pattern=[[1, 128]], channel_multiplier=-1,
                )
                nc.vector.memset(nbias[:], -EXPBIAS)

            def rope_compute(xEO, xT_ap, mul_eng, add_eng, cs=slice(0, S)):
                n = cs.stop - cs.start
                xE = xEO[:, 0:n]
                xO = xEO[:, n:2 * n]
                mul_eng.tensor_mul(xE, xE, FF[:, cs])
                mul_eng.tensor_mul(xO, xO, GG[:, cs])
                add_eng.tensor_add(xT_ap, xE, xO)

            def emit_load_boot():
                """Chunked load+RoPE for (b,h)=0 from the chunk-tiled boot
                tensors: qb0's matmuls start as soon as chunk 0 lands.
                Freqs load as two full-tensor DMAs (Scalar), v from the
                Tensor queue (own DMA ring, lands early), k chunks from
                Sync, q chunks from GpSimd; all RoPE on DVE."""
                # freqs in halves: chunk-0/1's [0:1024] slices land first
                # (the boot is HBM-bandwidth-bound across the three rings;
                # the priority set FFa+GGa+k0+q0 is 1MB, not 2MB).
                nc.scalar.dma_start(FF[:, 0:1024], fd.ap()[:, 0:1024])
                nc.scalar.dma_start(GG[:, 0:1024], gd.ap()[:, 0:1024])
                nc.scalar.dma_start(FF[:, 1024:S], fd.ap()[:, 1024:S])
                nc.scalar.dma_start(GG[:, 1024:S], gd.ap()[:, 1024:S])
                v_mm = iopool.tile([128, S], f16, tag="v_mm", name="v_mm")
                qTc = [xtpool.tile([128, 512], f16, tag=f"bqT{c}",
                                   name=f"bqT{c}", bufs=1) for c in range(4)]
                kTc = [xtpool.tile([128, 512], f16, tag=f"bkT{c}",
                                   name=f"bkT{c}", bufs=1) for c in range(4)]
                for c in range(4):
                    cs = slice(c * 512, (c + 1) * 512)
                    kEO = rpool.tile([128, 1024], f16, tag="bkEO",
                                     name="bkEO")
                    nc.sync.dma_start(kEO[:], kbd.ap()[c])
                    qEO = rpool.tile([128, 1024], f16, tag="bqEO",
                                     name="bqEO")
                    nc.gpsimd.dma_start(qEO[:], qbd.ap()[c])
                    rope_compute(kEO, kTc[c][:], nc.vector, nc.vector, cs)
                    rope_compute(qEO, qTc[c][:], nc.vector, nc.vector, cs)
                    if c == 0:
                        emit_consts()
                    if c == 1:
                        # v rides the Sync ring behind the first two k
                        # chunks; needed by the first PV (~after exp qb0).
                        nc.sync.dma_start(v_mm[:], vd.ap()[0])

                def kT_lhsT(tk):
                    return kTc[tk // 4][:, (tk % 4) * 128:(tk % 4 + 1) * 128]

                def qT_rhs(qb, off):
                    return qTc[qb][:, off:512]

                return (kT_lhsT, qT_rhs, v_mm)

            def emit_attention(bh, acc, hooks):
                kT_lhsT, qT_rhs, v_mm = acc
                last = bh == NBH - 1
                out_full = opool.tile([128, S], f16, tag="out_full",
                                      name="out_full")

                def phase_compute(qb):
                    nk, batches = _plan(qb)
                    last_tk = batches[-1][-1][0]
                    outs = out_full[:, qb * 512:(qb + 1) * 512]
                    outT = out_ps.tile([128, 512], f32, tag="outT",
                                       name="outT")
                    probts = []
                    na0 = sum(1 for t in batches[0] if t[1] == 0)
                    for batch in batches:
                        sc = sc_ps.tile([128, SCW], f32, tag="sc", name="sc")
                        for tk, off, lo, w in batch:
                            diag = tk >= 4 * qb
                            nc.tensor.matmul(
                                sc[:, lo:lo + w], kT_lhsT(tk),
                                qT_rhs(qb, off),
                                start=True, stop=not diag,
                            )
                            if diag:
                                nc.tensor.matmul(
                                    sc[:, lo:lo + 128],
                                    ident_bf[:], tri_bf[:],
                                    start=False, stop=True,
                                )
                        wtot = batch[-1][2] + batch[-1][3]
                        probt = ppool.tile([128, SCW], f16, tag="probt",
                                           name="probt")
                        nc.scalar.activation(
                            probt[:, 0:wtot], sc[:, 0:wtot],
                            Act.Exp, scale=SCALE, bias=nbias[:],
                        )
                        for tk, off, lo, w in batch:
                            nc.tensor.matmul(
                                outT[:, off:512],
                                v_mm[:, tk * 128:(tk + 1) * 128],
                                probt[:, lo:lo + w],
                                start=(tk == 0), stop=(tk == last_tk),
                            )
                        probts.append(probt)
                        # wide-tree merge: as soon as this batch's probt is
                        # final, add its q-aligned prefix (leading off==0
                        # tiles, always at 512-aligned lo) into batch 0's
                        # tile in place, [<=1536]-wide at DVE 2x rate.
                        if len(probts) > 1:
                            na = 0
                            for t in batch:
                                if t[1] != 0:
                                    break
                                na += 1
                            m = 512 * min(na, na0)
                            if m:
                                nc.vector.tensor_add(
                                    probts[0][:, 0:m], probts[0][:, 0:m],
                                    probt[:, 0:m])
                    # fold the misaligned diagonal tiles into batch 0's
                    # segment 0 at their q-offsets; the host folds the
                    # 512-col segments and reduces the 128 partitions.
                    p0 = probts[0]
                    for bi, batch in enumerate(batches):
                        seen_mis = False
                        for tk, off, lo, w in batch:
                            if off == 0 and not seen_mis:
                                continue
                            seen_mis = True
                            nc.vector.tensor_add(
                                p0[:, off:512], p0[:, off:512],
                                probts[bi][:, lo:lo + w])
                    nc.gpsimd.dma_start(
                        ad.ap()[bh, qb, :, 0:512 * na0],
                        p0[:, 0:512 * na0])
                    # evacuate the unnormalized PV accumulator fp32->fp16;
                    # host divides by the sums.  ScalarE (exp-bound) takes
                    # only one per (b,h); DVE the rest.
                    if qb == 0:
                        nc.scalar.copy(outs, outT[:])
                    else:
                        nc.vector.tensor_copy(outs, outT[:])
                    if last:
                        nc.gpsimd.dma_start(
                            od.ap()[bh, :, qb * 512:(qb + 1) * 512], outs)

                # last (b,h): large q-blocks first so the post-PE tail
                # (DVE tree + DMA) of the final block is the smallest one.
                order = (3, 2, 1, 0) if last else range(QB)
                for qb in order:
                    phase_compute(qb)
                    for fn in hooks.get(qb, ()):
                        fn()
                if not last:
                    nc.gpsimd.dma_start(od.ap()[bh], out_full[:])

            def make_hooks(bh, accs, state):
                """Stagger next-(b,h) loads and RoPE across this (b,h)'s
                q-blocks to smooth DMA/power bursts:
                  qb0 end: k load     qb1 end: q load + k RoPE muls (GpSimd)
                  qb2 end: k RoPE add + q RoPE (DVE) + v load
                (k load issues at attention start, before qb0.)"""
                nxt = bh + 1
                if nxt >= NBH:
                    return {}

                def load_k():
                    kEO = rpool.tile([128, 2 * S], f16, tag="kEO",
                                     name="kEO")
                    nc.sync.dma_start(kEO[:], kd.ap()[nxt])
                    state["kEO"] = kEO

                def load_q():
                    qEO = rpool.tile([128, 2 * S], f16, tag="qEO",
                                     name="qEO")
                    nc.sync.dma_start(qEO[:], qd.ap()[nxt])
                    state["qEO"] = qEO

                def rope_k_muls():
                    # all RoPE runs on DVE: a Pool/GpSimd tensor_tensor
                    # running concurrently with ANY DVE op slows the DVE
                    # op 10-20x (measured), so GpSimd gets no TT work.
                    kT = xtpool.tile([128, S], f16, tag="kT", name="kT")
                    kEO = state["kEO"]
                    nc.vector.tensor_mul(kEO[:, 0:S], kEO[:, 0:S], FF[:])
                    nc.vector.tensor_mul(kEO[:, S:2 * S], kEO[:, S:2 * S],
                                         GG[:])
                    state["kT"] = kT

                def rope_rest():
                    kEO, qEO = state["kEO"], state["qEO"]
                    kT, qT = state["kT"], xtpool.tile([128, S], f16,
                                                      tag="qT", name="qT")
                    nc.vector.tensor_add(kT[:], kEO[:, 0:S], kEO[:, S:2 * S])
                    rope_compute(qEO, qT[:], nc.vector, nc.vector)
                    v_mm = iopool.tile([128, S], f16, tag="v_mm",
                                       name="v_mm")
                    nc.sync.dma_start(v_mm[:], vd.ap()[nxt])

                    def kT_lhsT(tk):
                        return kT[:, tk * 128:(tk + 1) * 128]

                    def qT_rhs(qb, off):
                        return qT[:, qb * 512 + off:(qb + 1) * 512]

                    accs[nxt] = (kT_lhsT, qT_rhs, v_mm)

                if bh == 0:
                    # don't compete with the latency-critical boot chunks
                    return {0: (load_k,), 1: (load_q, rope_k_muls),
                            2: (rope_rest,)}
                return {-1: (load_k,), 0: (load_q,), 1: (rope_k_muls,),
                        2: (rope_rest,)}

            accs = {0: emit_load_boot()}

            for bh in range(NBH):
                state = {}
                hooks = make_hooks(bh, accs, state)
                for fn in hooks.get(-1, ()):
                    fn()
                emit_attention(bh, accs[bh], hooks)

    nc.compile()
    return nc


def _get_nc():
    if "nc" not in _CACHE:
        _CACHE["nc"] = _build()
    return _CACHE["nc"]


def _deint_T(x):
    # [N, S, D] -> de-interleave d (evens|odds) then transpose -> [N, D, S]
    return np.ascontiguousarray(
        np.concatenate([x[:, :, 0::2], x[:, :, 1::2]], axis=2)
        .transpose(0, 2, 1)).astype(np.float16)


def _fuse_eo(xT):
    # [N, D, S] (evens|odds on partitions) -> [N, 128, 2S] fused
    # (E|O) with E = (x0|x0), O = (x1|x1) duplicated across halves.
    E = np.concatenate([xT[:, 0:64], xT[:, 0:64]], axis=1)
    O = np.concatenate([xT[:, 64:128], xT[:, 64:128]], axis=1)
    return np.ascontiguousarray(np.concatenate([E, O], axis=2))


def _shard(q, k, v, freqs):
    q = np.asarray(q, dtype=np.float32)
    k = np.asarray(k, dtype=np.float32)
    v = np.asarray(v, dtype=np.float32)
    freqs = np.asarray(freqs, dtype=np.float32).reshape(S, D)

    fT = np.ascontiguousarray(
        np.concatenate([freqs[:, 0::2], freqs[:, 1::2]], axis=1)
        .T.astype(np.float16))
    gT = np.ascontiguousarray(
        np.concatenate([-freqs[:, 1::2], freqs[:, 0::2]], axis=1)
        .T.astype(np.float16))
    in_maps = []
    for c in range(NCORES):
        h0 = c * HPC

        def bhslice(x):
            # [B, S, Hc, D] -> [B, Hc, S, D] -> [NBH, S, D]
            return np.ascontiguousarray(
                x[:, :, h0:h0 + HPC, :].transpose(0, 2, 1, 3)
            ).reshape(NBH, S, D)

        # v s-tiled: vt[bh, p, t*128+d] = v[bh, t*128+p, d]
        vt = np.ascontiguousarray(
            bhslice(v).reshape(NBH, NT, 128, D).transpose(0, 2, 1, 3)
        ).reshape(NBH, 128, S).astype(np.float16)

        qT = _deint_T(bhslice(q))
        kT = _deint_T(bhslice(k))
        qEO = _fuse_eo(qT)
        kEO = _fuse_eo(kT)
        # boot chunks: [128, 1024] per 512-col chunk = (E_c | O_c)
        def _bctile(xEO):
            # [128, 2S] -> [4, 128, 1024]
            E = xEO[0, :, 0:S].reshape(128, 4, 512)
            O = xEO[0, :, S:2 * S].reshape(128, 4, 512)
            return np.ascontiguousarray(
                np.concatenate([E, O], axis=2).transpose(1, 0, 2))

        in_maps.append({
            "qEO": qEO,
            "kEO": kEO,
            "qEOb": _bctile(qEO),
            "kEOb": _bctile(kEO),
            "v": vt,
            "freqsT": fT,
            "freqsG": gT,
        })
    return in_maps


def kernel(q, k, v, freqs):
    nc = _get_nc()
    from concourse.bass_utils import run_bass_kernel_spmd

    in_maps = _shard(q, k, v, freqs)
    res = run_bass_kernel_spmd(nc, in_maps, core_ids=list(range(NCORES)))

    out = np.empty((B, H, S, D), dtype=np.float32)
    for c in range(NCORES):
        h0 = c * HPC
        # out dram [NBH, 128, S] = unnormalized outT[d, q].
        # accs dram [NBH, QB, 128, 1536]: per q-block, three 512-col
        # segments of partial k-tile sums (qb0: only segment 0 valid);
        # sums[q] = sum over segments and partitions.
        oT = res.results[c]["out"].astype(np.float32)       # [NBH, D, S]
        ac = res.results[c]["accs"].astype(np.float32)      # [NBH,QB,128,1536]
        sums = ac[:, :, :, 0:512].sum(axis=2)               # [NBH, QB, 512]
        sums[:, 1:] += ac[:, 1:, :, 512:1024].sum(axis=2)
        sums[:, 1:] += ac[:, 1:, :, 1024:1536].sum(axis=2)
        sums = sums.reshape(NBH, S)
        out[:, h0:h0 + HPC] = (
            oT / sums[:, None, :]).transpose(0, 2, 1).reshape(B, HPC, S, D)
    return out


# revision 19
# speedup vs baseline: 1.2865x; 1.0131x over previous
"""Trainium2 Bass kernel: Wan-style interleaved RoPE on q/k + causal attention.

Full problem: q,k,v [B=2, S=2048, H=16, D=128] fp32, freqs [1, S, 1, D].
  rq = rope(q), rk = rope(k)
  out[b,h,q,d] = softmax_causal(rq @ rk^T / sqrt(D)) @ v      -> [B, H, S, D]

Sharding: heads split across 8 cores (2 heads/core); each core handles
4 independent (b, h) attention problems. Inputs are sliced on host, the
SPMD kernel runs on cores 0-7, outputs are concatenated on host.

Layout trick: scores = sum_d rq[d]*rk[d] are invariant under any shared
permutation of d, so q and k are shipped de-interleaved (evens then
odds), pre-transposed to [D', S], AND pre-duplicated: the vector
engines have no cross-partition path, so each shipped tensor is
[128, 2S] = (E | O) with E = (x0|x0), O = (x1|x1) stacked so that with
FF = (f0|f1), GG = (-f1|f0):
  rqT' = E*FF + O*GG    -- exactly interleaved RoPE in (evens|odds)
                           d-order.
One fused DMA loads (E|O) per tensor (same bytes as the four half-DMAs
it replaces -- the duplication is in DRAM -- but 4x fewer descriptors).

Everything is shipped and computed in fp16.  Attention per (b,h), per
q-block of 512: k-tiles are bin-packed into 512-col PSUM banks, three
banks per [128, 1536] score tile, diagonal tiles reordered j0,j1,j3,j2
so the packing has no holes; one exp(s*scale - 8) per batch (the
uniform bias cancels in the normalization and keeps exp in fp16 range).

Softmax sums and normalization are finished on the HOST: the kernel
merges each q-block's probT tiles with an in-place wide tree on DVE --
q-aligned full batches are added [1536]-wide at 2x perf mode into batch
0's tile, misaligned diagonal tiles are added at their q-offsets into
segment 0 -- and ships batch 0 raw ([128,1536]; [128,512] for qb0).
The host folds the 512-col segments, reduces the 128 partitions,
and divides the unnormalized output (outT evacuated fp32->fp16 by
ScalarE into out_full [d, q]) by the sums.

Engine balance: q RoPE (2 mul + add) and k's add on DVE; k's two muls
on GpSimd; exp + outT evac on ScalarE; out/acc DMAs issue from GpSimd,
loads from Sync.  Load DMAs are staggered (k at attention start, q
after qb0, v after qb2) and the next (b,h)'s RoPE is emitted in two
stages (k muls after qb1, the DVE ops after qb2 -- so the DVE add
gated on GpSimd never blocks the in-order vector queue) to smooth the
DMA/power bursts that otherwise stall DVE ops 10-20x.

Boot: (b,h)=0's q/k are shipped a second time chunk-tiled [4, 128,
1024] ((E|O) per 512-col chunk, contiguous in DRAM) so boot loads+RoPE
run chunk-by-chunk and qb0's matmuls start early; freqs/v issue from
the otherwise-idle Scalar queue.
"""

import math

import numpy as np

B, S, H, D = 2, 2048, 16, 128
NCORES = 8
HPC = H // NCORES          # heads per core
NBH = B * HPC              # (b, h) problems per core
NT = S // 128              # s-tiles
QB = S // 512              # q blocks of 512
SCALE = 1.0 / math.sqrt(D)
NEG = -1e30
EXPBIAS = 8.0              # uniform softmax shift; keeps exp in fp16 range
SCW = 1536                 # packed score-tile width (3 PSUM banks)

_CACHE = {}


def _plan(qb):
    """Pack this q-block's k-tiles into contiguous score batches.

    A matmul output must not cross a 512-col PSUM bank boundary, so tiles
    are bin-packed into 512-col banks (3 banks per [128, SCW] score
    tile).  The diagonal tiles (widths 512/384/256/128) are emitted in
    the order j0, j1, j3, j2 so banks fill exactly ([512], [384+128],
    [256]) with no holes: each batch's valid columns are contiguous from
    0 and one exp instruction covers them.  The first tile (tk=0, full
    width) stays first so its start=True matmul resets every PSUM cell
    of the PV accumulator.

    Returns (nk, batches); each batch is a list of (tk, off, lo, w).
    Tiles with off == 0 always land at 512-aligned lo (bank starts), so
    the leading off==0 run of every batch is q-aligned for wide adds.
    """
    nk = 4 * qb + 4
    order = list(range(4 * qb)) + [4 * qb, 4 * qb + 1, 4 * qb + 3, 4 * qb + 2]
    batches, cur = [], []
    bank, used = 0, 0
    for tk in order:
        j = tk - 4 * qb
        off = 128 * j if j > 0 else 0
        w = 512 - off
        if used + w > 512:
            bank, used = bank + 1, 0
        if bank == SCW // 512:
            batches.append(cur)
            cur, bank = [], 0
        cur.append((tk, off, bank * 512 + used, w))
        used += w
    batches.append(cur)
    return nk, batches


def _build():
    import concourse.mybir as mybir
    import concourse.tile as tile
    from concourse import bacc
    from concourse.masks import make_identity

    f32 = mybir.dt.float32
    f16 = mybir.dt.float16
    bf16 = mybir.dt.bfloat16
    Alu = mybir.AluOpType
    Act = mybir.ActivationFunctionType

    nc = bacc.Bacc("TRN2", target_bir_lowering=False, debug=False,
                   num_devices=NCORES)
    # steady-state loads: fused (E|O) [128, 2S] per (b,h)
    qd = nc.dram_tensor("qEO", [NBH, 128, 2 * S], f16, kind="ExternalInput")
    kd = nc.dram_tensor("kEO", [NBH, 128, 2 * S], f16, kind="ExternalInput")
    vd = nc.dram_tensor("v", [NBH, 128, S], f16, kind="ExternalInput")
    # boot copies of (b,h)=0's q/k, chunk-tiled [4, 128, 1024] ((E|O) per
    # 512-col chunk, contiguous): boot loads+RoPE run chunk-by-chunk.
    qbd = nc.dram_tensor("qEOb", [4, 128, 1024], f16, kind="ExternalInput")
    kbd = nc.dram_tensor("kEOb", [4, 128, 1024], f16, kind="ExternalInput")
    fd = nc.dram_tensor("freqsT", [D, S], f16, kind="ExternalInput")
    gd = nc.dram_tensor("freqsG", [D, S], f16, kind="ExternalInput")
    od = nc.dram_tensor("out", [NBH, 128, S], f16, kind="ExternalOutput")
    ad = nc.dram_tensor("accs", [NBH, QB, 128, SCW], f16,
                        kind="ExternalOutput")

    with tile.TileContext(nc) as tc:
        with (
            tc.tile_pool(name="const", bufs=1) as cpool,
            tc.tile_pool(name="io", bufs=2) as iopool,
            tc.tile_pool(name="rope", bufs=2) as rpool,
            tc.tile_pool(name="xt", bufs=2) as xtpool,
            tc.tile_pool(name="prob", bufs=9) as ppool,
            tc.tile_pool(name="outf", bufs=2) as opool,
            tc.tile_pool(name="sc_ps", bufs=2, space="PSUM") as sc_ps,
            tc.tile_pool(name="out_ps", bufs=2, space="PSUM") as out_ps,
        ):
            # ---- constants (tiles only; ops are emitted mid-boot so the
            # latency-critical chunk-0 loads+RoPE go first on each queue) --
            tri_bf = cpool.tile([128, 128], bf16, tag="tri_bf")
            ident_bf = cpool.tile([128, 128], bf16, tag="ident_bf")
            nbias = cpool.tile([128, 1], f32, tag="nbias")
            FF = cpool.tile([128, S], f16, tag="FF")
            GG = cpool.tile([128, S], f16, tag="GG")

            def emit_consts():
                make_identity(nc, ident_bf[:])
                # tri_bf[k, t] = 0 where k <= t (valid), NEG where k > t.
                nc.gpsimd.memset(tri_bf[:], 0.0)
                nc.gpsimd.affine_select(
                    out=tri_bf[:], in_=tri_bf[:],
                    compare_op=Alu.is_ge, fill=NEG, base=0,
                    pattern=[[1, 128]], channel_multiplier=-1,
                )
                nc.vector.memset(nbias[:], -EXPBIAS)

            def rope_compute(xEO, xT_ap, mul_eng, add_eng, cs=slice(0, S)):
                n = cs.stop - cs.start
                xE = xEO[:, 0:n]
                xO = xEO[:, n:2 * n]
                mul_eng.tensor_mul(xE, xE, FF[:, cs])
                mul_eng.tensor_mul(xO, xO, GG[:, cs])
                add_eng.tensor_add(xT_ap, xE, xO)

            def emit_load_boot():
                """Chunked load+RoPE for (b,h)=0 from the chunk-tiled boot
                tensors: qb0's matmuls start as soon as chunk 0 lands.
                Freqs load as two full-tensor DMAs (Scalar), v from the
                Tensor queue (own DMA ring, lands early), k chunks from
                Sync, q chunks from GpSimd; all RoPE on DVE."""
                # freqs in halves: chunk-0/1's [0:1024] slices land first
                # (the boot is HBM-bandwidth-bound across the three rings;
                # the priority set FFa+GGa+k0+q0 is 1MB, not 2MB).
                nc.scalar.dma_start(FF[:, 0:1024], fd.ap()[:, 0:1024])
                nc.scalar.dma_start(GG[:, 0:1024], gd.ap()[:, 0:1024])
                nc.scalar.dma_start(FF[:, 1024:S], fd.ap()[:, 1024:S])
                nc.scalar.dma_start(GG[:, 1024:S], gd.ap()[:, 1024:S])
                v_mm = iopool.tile([128, S], f16, tag="v_mm", name="v_mm")
                qTc = [xtpool.tile([128, 512], f16, tag=f"bqT{c}",
                                   name=f"bqT{c}", bufs=1) for c in range(4)]
                kTc = [xtpool.tile([128, 512], f16, tag=f"bkT{c}",
                                   name=f"bkT{c}", bufs=1) for c in range(4)]
                for c in range(4):
                    cs = slice(c * 512, (c + 1) * 512)
                    kEO = rpool.tile([128, 1024], f16, tag="bkEO",
                                     name="bkEO")
                    nc.sync.dma_start(kEO[:], kbd.ap()[c])
                    qEO = rpool.tile([128, 1024], f16, tag="bqEO",
                                     name="bqEO")
                    nc.gpsimd.dma_start(qEO[:], qbd.ap()[c])
                    rope_compute(kEO, kTc[c][:], nc.vector, nc.vector, cs)
                    rope_compute(qEO, qTc[c][:], nc.vector, nc.vector, cs)
                    if c == 0:
                        emit_consts()
                    if c == 1:
                        # v rides the Sync ring behind the first two k
                        # chunks; needed by the first PV (~after exp qb0).
                        nc.sync.dma_start(v_mm[:], vd.ap()[0])

                def kT_lhsT(tk):
                    return kTc[tk // 4][:, (tk % 4) * 128:(tk % 4 + 1) * 128]

                def qT_rhs(qb, off):
                    return qTc[qb][:, off:512]

                return (kT_lhsT, qT_rhs, v_mm)

            def emit_attention(bh, acc, hooks):
                kT_lhsT, qT_rhs, v_mm = acc
                last = bh == NBH - 1
                out_full = opool.tile([128, S], f16, tag="out_full",
                                      name="out_full")

                pending = [None]

                def phase_compute(qb):
                    nk, batches = _plan(qb)
                    nb = len(batches)
                    last_tk = batches[-1][-1][0]
                    outs = out_full[:, qb * 512:(qb + 1) * 512]
                    outT = out_ps.tile([128, 512], f32, tag="outT",
                                       name="outT")
                    probts = []
                    na0 = sum(1 for t in batches[0] if t[1] == 0)

                    def emit_pv(batch, probt):
                        for tk, off, lo, w in batch:
                            nc.tensor.matmul(
                                outT[:, off:512],
                                v_mm[:, tk * 128:(tk + 1) * 128],
                                probt[:, lo:lo + w],
                                start=(tk == 0), stop=(tk == last_tk),
                            )

                    def emit_tree_add(batch, probt):
                        # wide-tree merge: add this batch's q-aligned prefix
                        # (leading off==0 tiles, always at 512-aligned lo)
                        # into batch 0's tile in place, [<=1536]-wide at DVE
                        # 2x rate.
                        na = 0
                        for t in batch:
                            if t[1] != 0:
                                break
                            na += 1
                        m = 512 * min(na, na0)
                        if m:
                            nc.vector.tensor_add(
                                probts[0][:, 0:m], probts[0][:, 0:m],
                                probt[:, 0:m])

                    for bi, batch in enumerate(batches):
                        sc = sc_ps.tile([128, SCW], f32, tag="sc", name="sc")
                        for tk, off, lo, w in batch:
                            diag = tk >= 4 * qb
                            nc.tensor.matmul(
                                sc[:, lo:lo + w], kT_lhsT(tk),
                                qT_rhs(qb, off),
                                start=True, stop=not diag,
                            )
                            if diag:
                                nc.tensor.matmul(
                                    sc[:, lo:lo + 128],
                                    ident_bf[:], tri_bf[:],
                                    start=False, stop=True,
                                )
                        wtot = batch[-1][2] + batch[-1][3]
                        probt = ppool.tile([128, SCW], f16, tag="probt",
                                           name="probt")
                        nc.scalar.activation(
                            probt[:, 0:wtot], sc[:, 0:wtot],
                            Act.Exp, scale=SCALE, bias=nbias[:],
                        )
                        probts.append(probt)
                        if bi == 0 and pending[0] is not None:
                            # finish the PREVIOUS q-block here: its final
                            # (diagonal) batch's PVs wait on that batch's
                            # exp, so emitting them after this q-block's
                            # batch-0 scores+exp keeps the in-order PE
                            # queue from stalling on the exp.
                            pending[0]()
                            pending[0] = None
                        if bi < nb - 1:
                            emit_pv(batch, probt)
                            if bi > 0:
                                emit_tree_add(batch, probt)

                    def finish(batch=batches[-1], probt=probts[-1]):
                        emit_pv(batch, probt)
                        if nb > 1:
                            emit_tree_add(batch, probt)
                        # fold the misaligned diagonal tiles into batch 0's
                        # segment 0 at their q-offsets; the host folds the
                        # 512-col segments and reduces the 128 partitions.
                        p0 = probts[0]
                        for bi, b in enumerate(batches):
                            seen_mis = False
                            for tk, off, lo, w in b:
                                if off == 0 and not seen_mis:
                                    continue
                                seen_mis = True
                                nc.vector.tensor_add(
                                    p0[:, off:512], p0[:, off:512],
                                    probts[bi][:, lo:lo + w])
                        nc.gpsimd.dma_start(
                            ad.ap()[bh, qb, :, 0:512 * na0],
                            p0[:, 0:512 * na0])
                        # evacuate the unnormalized PV accumulator
                        # fp32->fp16; host divides by the sums.  ScalarE
                        # (exp-bound) takes one per (b,h); DVE the rest.
                        if qb == 0:
                            nc.scalar.copy(outs, outT[:])
                        else:
                            nc.vector.tensor_copy(outs, outT[:])
                        if last:
                            nc.gpsimd.dma_start(
                                od.ap()[bh, :, qb * 512:(qb + 1) * 512],
                                outs)

                    pending[0] = finish

                # last (b,h): large q-blocks first so the post-PE tail
                # (DVE tree + DMA) of the final block is the smallest one.
                order = (3, 2, 1, 0) if last else range(QB)
                for qb in order:
                    phase_compute(qb)
                    for fn in hooks.get(qb, ()):
                        fn()
                if pending[0] is not None:
                    pending[0]()
                if not last:
                    nc.gpsimd.dma_start(od.ap()[bh], out_full[:])

            def make_hooks(bh, accs, state):
                """Stagger next-(b,h) loads and RoPE across this (b,h)'s
                q-blocks to smooth DMA/power bursts:
                  qb0 end: k load     qb1 end: q load + k RoPE muls (GpSimd)
                  qb2 end: k RoPE add + q RoPE (DVE) + v load
                (k load issues at attention start, before qb0.)"""
                nxt = bh + 1
                if nxt >= NBH:
                    return {}

                def load_k():
                    kEO = rpool.tile([128, 2 * S], f16, tag="kEO",
                                     name="kEO")
                    nc.sync.dma_start(kEO[:], kd.ap()[nxt])
                    state["kEO"] = kEO

                def load_q():
                    qEO = rpool.tile([128, 2 * S], f16, tag="qEO",
                                     name="qEO")
                    nc.sync.dma_start(qEO[:], qd.ap()[nxt])
                    state["qEO"] = qEO

                def rope_k_muls():
                    # all RoPE runs on DVE: a Pool/GpSimd tensor_tensor
                    # running concurrently with ANY DVE op slows the DVE
                    # op 10-20x (measured), so GpSimd gets no TT work.
                    kT = xtpool.tile([128, S], f16, tag="kT", name="kT")
                    kEO = state["kEO"]
                    nc.vector.tensor_mul(kEO[:, 0:S], kEO[:, 0:S], FF[:])
                    nc.vector.tensor_mul(kEO[:, S:2 * S], kEO[:, S:2 * S],
                                         GG[:])
                    state["kT"] = kT

                def rope_rest():
                    kEO, qEO = state["kEO"], state["qEO"]
                    kT, qT = state["kT"], xtpool.tile([128, S], f16,
                                                      tag="qT", name="qT")
                    nc.vector.tensor_add(kT[:], kEO[:, 0:S], kEO[:, S:2 * S])
                    rope_compute(qEO, qT[:], nc.vector, nc.vector)
                    v_mm = iopool.tile([128, S], f16, tag="v_mm",
                                       name="v_mm")
                    nc.sync.dma_start(v_mm[:], vd.ap()[nxt])

                    def kT_lhsT(tk):
                        return kT[:, tk * 128:(tk + 1) * 128]

                    def qT_rhs(qb, off):
                        return qT[:, qb * 512 + off:(qb + 1) * 512]

                    accs[nxt] = (kT_lhsT, qT_rhs, v_mm)

                if bh == 0:
                    # don't compete with the latency-critical boot chunks
                    return {0: (load_k,), 1: (load_q, rope_k_muls),
                            2: (rope_rest,)}
                return {-1: (load_k,), 0: (load_q,), 1: (rope_k_muls,),
                        2: (rope_rest,)}

            accs = {0: emit_load_boot()}

            for bh in range(NBH):
                state = {}
                hooks = make_hooks(bh, accs, state)
                for fn in hooks.get(-1, ()):
                    fn()
                emit_attention(bh, accs[bh], hooks)

    nc.compile()
    return nc


def _get_nc():
    if "nc" not in _CACHE:
        _CACHE["nc"] = _build()
    return _CACHE["nc"]


def _deint_T(x):
    # [N, S, D] -> de-interleave d (evens|odds) then transpose -> [N, D, S]
    return np.ascontiguousarray(
        np.concatenate([x[:, :, 0::2], x[:, :, 1::2]], axis=2)
        .transpose(0, 2, 1)).astype(np.float16)


def _fuse_eo(xT):
    # [N, D, S] (evens|odds on partitions) -> [N, 128, 2S] fused
    # (E|O) with E = (x0|x0), O = (x1|x1) duplicated across halves.
    E = np.concatenate([xT[:, 0:64], xT[:, 0:64]], axis=1)
    O = np.concatenate([xT[:, 64:128], xT[:, 64:128]], axis=1)
    return np.ascontiguousarray(np.concatenate([E, O], axis=2))


def _shard(q, k, v, freqs):
    q = np.asarray(q, dtype=np.float32)
    k = np.asarray(k, dtype=np.float32)
    v = np.asarray(v, dtype=np.float32)
    freqs = np.asarray(freqs, dtype=np.float32).reshape(S, D)

    fT = np.ascontiguousarray(
        np.concatenate([freqs[:, 0::2], freqs[:, 1::2]], axis=1)
        .T.astype(np.float16))
    gT = np.ascontiguousarray(
        np.concatenate([-freqs[:, 1::2], freqs[:, 0::2]], axis=1)
        .T.astype(np.float16))
    in_maps = []
    for c in range(NCORES):
        h0 = c * HPC

        def bhslice(x):
            # [B, S, Hc, D] -> [B, Hc, S, D] -> [NBH, S, D]
            return np.ascontiguousarray(
                x[:, :, h0:h0 + HPC, :].transpose(0, 2, 1, 3)
            ).reshape(NBH, S, D)

        # v s-tiled: vt[bh, p, t*128+d] = v[bh, t*128+p, d]
        vt = np.ascontiguousarray(
            bhslice(v).reshape(NBH, NT, 128, D).transpose(0, 2, 1, 3)
        ).reshape(NBH, 128, S).astype(np.float16)

        qT = _deint_T(bhslice(q))
        kT = _deint_T(bhslice(k))
        qEO = _fuse_eo(qT)
        kEO = _fuse_eo(kT)
        # boot chunks: [128, 1024] per 512-col chunk = (E_c | O_c)
        def _bctile(xEO):
            # [128, 2S] -> [4, 128, 1024]
            E = xEO[0, :, 0:S].reshape(128, 4, 512)
            O = xEO[0, :, S:2 * S].reshape(128, 4, 512)
            return np.ascontiguousarray(
                np.concatenate([E, O], axis=2).transpose(1, 0, 2))

        in_maps.append({
            "qEO": qEO,
            "kEO": kEO,
            "qEOb": _bctile(qEO),
            "kEOb": _bctile(kEO),
            "v": vt,
            "freqsT": fT,
            "freqsG": gT,
        })
    return in_maps


def kernel(q, k, v, freqs):
    nc = _get_nc()
    from concourse.bass_utils import run_bass_kernel_spmd

    in_maps = _shard(q, k, v, freqs)
    res = run_bass_kernel_spmd(nc, in_maps, core_ids=list(range(NCORES)))

    out = np.empty((B, H, S, D), dtype=np.float32)
    for c in range(NCORES):
        h0 = c * HPC
        # out dram [NBH, 128, S] = unnormalized outT[d, q].
        # accs dram [NBH, QB, 128, 1536]: per q-block, three 512-col
        # segments of partial k-tile sums (qb0: only segment 0 valid);
        # sums[q] = sum over segments and partitions.
        oT = res.results[c]["out"].astype(np.float32)       # [NBH, D, S]
        ac = res.results[c]["accs"].astype(np.float32)      # [NBH,QB,128,1536]
        sums = ac[:, :, :, 0:512].sum(axis=2)               # [NBH, QB, 512]
        sums[:, 1:] += ac[:, 1:, :, 512:1024].sum(axis=2)
        sums[:, 1:] += ac[:, 1:, :, 1024:1536].sum(axis=2)
        sums = sums.reshape(NBH, S)
        out[:, h0:h0 + HPC] = (
            oT / sums[:, None, :]).transpose(0, 2, 1).reshape(B, HPC, S, D)
    return out
